# revision 1
# baseline (speedup 1.0000x reference)
"""AxialMambaBlock on 8 Trainium2 NeuronCores (Bass/Tile).

Sharding: data-parallel over the folded sequence-batch axis. Each mamba
processes 112 sequences of length 56; each core takes 14 sequences of the
height-mamba and 14 of the width-mamba. Host does tiny weight fusion +
final gather/add.

Scan math: reference computes x_t = num_t/(c_t+1e-6) with
num_t = sum_{j<=t} dBu_j c_j, c_t = exp(sum_{j>t} dA_j).  Exactly:
x_t = g_t * h_t with h the standard recurrence h_t = exp(dA_t) h_{t-1}+dBu_t
and g_t = sigmoid(s_t + ln(1e6)), s_t = sum_{j>t} dA_j.  (dA<0 always, so
the reference's min(.,15) clamp never fires.)

Layout: d (internal dim, 192) splits into a 128-row head and a 64-row
tail; the tail processes TWO sequences per instruction (rows 0:64 = seq s,
rows 64:128 = seq s+1 via shifted-duplicate operand tiles), so each
mamba runs 21 instead of 28 [128, n*t]=[128, 5376] scan blocks.  The
hardware tensor_tensor_scan runs along t (t-inner layout, per-sequence
resets via a_0 := 0).  Elementwise ops use raw TENSOR_TENSOR in bf16
(2x packed DVE mode, including broadcast-middle-dim operands); B/C are
replicated across partitions by gpsimd partition_broadcast (head) or K=1
TensorE matmuls (tail pairs, which need different data per row half),
fed from DMA-flattened per-sequence rows.  The n-reduction is an
in-place binary tree of TT adds.  exp/tanh run on ACT (one table set;
sigmoid g folded to (1+tanh)/2 with the 0.5 folded into C on host);
av production is software-pipelined one block ahead.
"""

import os
import sys

import numpy as np

for _p in ("/opt/trn_rl_repo", "/root/.axon_site/_ro/trn_rl_repo"):
    if os.path.isdir(_p) and _p not in sys.path:
        sys.path.append(_p)

D_IN = 96
D_INT = 192
NST = 96          # state dim n
DTR = 6
KCV = 4           # conv taps
BN_EPS = 1e-5
N_CORES = 8
B = 2
CIO = 64
HH = 56
WW = 56
L = 56            # sequence length
SPC = 14          # sequences per core per mamba
TOK = SPC * L     # 784 tokens per core per mamba
PITCH = 60        # padded per-seq pitch for conv shifts
PADC = 4 + SPC * PITCH   # 844
NCHUNK = 32       # n-values per replication chunk (32*56=1792)
BIG = NST * L     # 5376
LN1E6 = 13.815510557964274

LAST_HW_EXEC_NS = None

_CACHE = {}


def _build_bass():
    import concourse.bacc as bacc
    import concourse.mybir as mybir
    import concourse.tile as tile

    dt = mybir.dt
    f32 = dt.float32
    bf16 = dt.bfloat16
    Alu = mybir.AluOpType
    Act = mybir.ActivationFunctionType

    nc = bacc.Bacc("TRN2", target_bir_lowering=False, debug=False,
                   num_devices=N_CORES)

    def tt(out, in0, in1, op):
        return nc.vector.add_instruction(mybir.InstTensorTensor(
            name=nc.get_next_instruction_name(), op=op,
            ins=[nc.vector.lower_ap(in0), nc.vector.lower_ap(in1)],
            outs=[nc.vector.lower_ap(out)]))

    # ---- DRAM I/O ----
    dram_in = {}

    def din(name, shape):
        dram_in[name] = nc.dram_tensor(name, list(shape), f32,
                                       kind="ExternalInput").ap()

    for m in ("h", "w"):
        din(f"tokT_{m}", (CIO, TOK))
        din(f"winT_{m}", (CIO, 4 * D_IN))      # fused (in_w@down).T
        din(f"convw_{m}", (D_INT, KCV))
        din(f"convb_{m}", (D_INT, 1))
        din(f"wdT_{m}", (D_INT, D_INT))        # (dproj@xproj[:6]).T
        din(f"dpb_{m}", (D_INT, 1))
        din(f"xbcT_{m}", (D_INT, 2 * NST))     # xproj[6:].T  [B|C]
        din(f"A_{m}", (D_INT, NST))            # -exp(clip(A_log))
        din(f"Dp_{m}", (D_INT, 1))
        din(f"woutT_{m}", (D_INT, CIO))        # (bn_inv*(up@out_w)).T
        din(f"bout_{m}", (CIO, 1))

    dram_out = {
        "h": nc.dram_tensor("out_h", [CIO, TOK], f32,
                            kind="ExternalOutput").ap(),
        "w": nc.dram_tensor("out_w", [CIO, TOK], f32,
                            kind="ExternalOutput").ap(),
    }

    HSP, TSP = 128, 64          # d split: head rows / tail rows
    with tile.TileContext(nc) as tc:
        with (
            tc.tile_pool(name="wts", bufs=1) as wts,
            tc.tile_pool(name="sm", bufs=1) as sm,
            tc.tile_pool(name="big", bufs=1) as big,
            tc.tile_pool(name="big2", bufs=1) as big2,
            tc.tile_pool(name="psA", bufs=1, space="PSUM") as psA,
            tc.tile_pool(name="psB", bufs=2, space="PSUM") as psB,
        ):
            # constant helper tiles
            ones1 = wts.tile([1, HSP], bf16, name="ones1")
            nc.gpsimd.memset(ones1[:], 1.0)
            ln1e6 = wts.tile([HSP, 1], f32, name="ln1e6")
            nc.gpsimd.memset(ln1e6[:], LN1E6 / 2.0)
            one_b = wts.tile([HSP, 1], f32, name="one_b")
            nc.gpsimd.memset(one_b[:], 1.0)
            # cumsum gate: 1 everywhere, 0 at t=0 of each sequence
            gate = wts.tile([HSP, TOK], bf16, name="gate")
            nc.gpsimd.memset(gate[:], 1.0)
            g3 = gate.rearrange("p (s t) -> p s t", t=L)
            nc.gpsimd.memset(g3[:, :, 0:1], 0.0)

            ROWS = (HSP, TSP)

            def halved(name_base, m, cols):
                """Load a [D_INT, cols] DRAM tensor as [128,cols]+[64,cols]."""
                out = []
                for hf in range(2):
                    r0 = hf * HSP
                    t = wts.tile([ROWS[hf], cols], f32,
                                 name=f"{name_base}{hf}_{m}",
                                 tag=f"{name_base}{hf}")
                    nc.sync.dma_start(
                        t[:], dram_in[f"{name_base}_{m}"][r0:r0 + ROWS[hf], :])
                    out.append(t)
                return out

            for m in ("h", "w"):
                # ---------- load weights ----------
                tokT = wts.tile([CIO, TOK], f32, name=f"tokT_{m}", tag="tokT")
                nc.sync.dma_start(tokT[:], dram_in[f"tokT_{m}"][:])
                winT = wts.tile([CIO, 4 * D_IN], f32, name=f"winT_{m}",
                                tag="winT")
                nc.sync.dma_start(winT[:], dram_in[f"winT_{m}"][:])
                convw = halved("convw", m, KCV)
                convb = halved("convb", m, 1)
                wdT = halved("wdT", m, D_INT)
                dpb = halved("dpb", m, 1)
                xbcT = halved("xbcT", m, 2 * NST)
                Amat = halved("A", m, NST)
                Dp = halved("Dp", m, 1)
                woutT = halved("woutT", m, CIO)
                bout = wts.tile([CIO, 1], f32, name=f"bout_{m}", tag="bout")
                nc.sync.dma_start(bout[:], dram_in[f"bout_{m}"][:])
                # A for the tail-pair block: tail rows duplicated (no shift)
                Adup = wts.tile([HSP, NST], f32, name=f"Adup_{m}", tag="Adup")
                nc.sync.dma_start(Adup[0:TSP, :], dram_in[f"A_{m}"][HSP:, :])
                nc.sync.dma_start(Adup[TSP:, :], dram_in[f"A_{m}"][HSP:, :])

                # ---------- in-projection (fused down-proj) ----------
                FCH = ((0, 0), (1, HSP), (2, D_INT), (3, D_INT + HSP))
                x1pad, x1s, res_s = [None, None], [None, None], [None, None]
                for hf in range(2):
                    xp = sm.tile([ROWS[hf], PADC], f32, name=f"x1pad{hf}_{m}",
                                 tag=f"x1pad{hf}")
                    nc.gpsimd.memset(xp[:], 0.0)
                    x1pad[hf] = xp
                for fc in range(4):
                    hf = fc % 2
                    col0 = FCH[fc][1]
                    rows = ROWS[hf]
                    ps = psA.tile([rows, TOK], f32, name=f"psin{fc}_{m}",
                                  tag=f"psA{hf}")
                    for c0 in range(0, TOK, 512):
                        c1 = min(c0 + 512, TOK)
                        nc.tensor.matmul(ps[:, c0:c1],
                                         winT[:, col0:col0 + rows],
                                         tokT[:, c0:c1],
                                         start=True, stop=True)
                    if fc < 2:
                        dst = x1pad[hf][:, 4:4 + SPC * PITCH].rearrange(
                            "p (s t) -> p s t", t=PITCH)[:, :, 0:L]
                        nc.scalar.copy(dst,
                                       ps.rearrange("p (s t) -> p s t", t=L))
                    else:
                        rs = sm.tile([rows, TOK], bf16, name=f"res{hf}_{m}",
                                     tag=f"res{hf}")
                        nc.scalar.activation(rs[:], ps[:], Act.Silu)
                        res_s[hf] = rs

                # ---------- depthwise causal conv + SiLU ----------
                for hf in range(2):
                    rows = ROWS[hf]
                    ca = sm.tile([rows, TOK], f32, name=f"ca{hf}_{m}",
                                 tag=f"ca{hf}")
                    cb = sm.tile([rows, TOK], f32, name=f"cb{hf}_{m}",
                                 tag=f"cb{hf}")

                    def tap(k, _hf=hf):
                        return x1pad[_hf][:, 1 + k:1 + k +
                                          SPC * PITCH].rearrange(
                            "p (s t) -> p s t", t=PITCH)[:, :, 0:L]

                    ca3 = ca.rearrange("p (s t) -> p s t", t=L)
                    cb3 = cb.rearrange("p (s t) -> p s t", t=L)
                    nc.vector.tensor_scalar_mul(ca3, tap(0), convw[hf][:, 0:1])
                    nc.vector.scalar_tensor_tensor(cb3, tap(1),
                                                   convw[hf][:, 1:2], ca3,
                                                   op0=Alu.mult, op1=Alu.add)
                    nc.vector.scalar_tensor_tensor(ca3, tap(2),
                                                   convw[hf][:, 2:3], cb3,
                                                   op0=Alu.mult, op1=Alu.add)
                    nc.vector.scalar_tensor_tensor(cb3, tap(3),
                                                   convw[hf][:, 3:4], ca3,
                                                   op0=Alu.mult, op1=Alu.add)
                    xs = sm.tile([rows, TOK], f32, name=f"x1s{hf}_{m}",
                                 tag=f"x1s{hf}")
                    nc.scalar.activation(xs[:], cb[:], Act.Silu,
                                         bias=convb[hf][:])
                    x1s[hf] = xs

                # ---------- x_dbl: delta / B / C ----------
                delta, Sd, Pdu = [None, None], [None, None], [None, None]
                for hf in range(2):
                    rows = ROWS[hf]
                    ps = psA.tile([rows, TOK], f32, name=f"psd{hf}_{m}",
                                  tag=f"psA{hf}")
                    col0 = hf * HSP
                    for c0 in range(0, TOK, 512):
                        c1 = min(c0 + 512, TOK)
                        nc.tensor.matmul(ps[:, c0:c1],
                                         wdT[0][:, col0:col0 + rows],
                                         x1s[0][:, c0:c1],
                                         start=True, stop=False)
                        nc.tensor.matmul(ps[:, c0:c1],
                                         wdT[1][:, col0:col0 + rows],
                                         x1s[1][:, c0:c1],
                                         start=False, stop=True)
                    dl = sm.tile([rows, TOK], bf16, name=f"delta{hf}_{m}",
                                 tag=f"delta{hf}")
                    dtmp = sm.tile([rows, TOK], f32, name=f"dtmp{hf}_{m}",
                                   tag=f"P{hf}")
                    nc.vector.tensor_scalar_min(dtmp[:], ps[:], 30.0)
                    nc.scalar.activation(dl[:], dtmp[:], Act.Exp,
                                         bias=dpb[hf][:])
                    nc.vector.tensor_scalar_add(dtmp[:], dl[:], 1.0)
                    nc.scalar.activation(dl[:], dtmp[:], Act.Ln)
                    delta[hf] = dl

                Bsb = sm.tile([NST, TOK], bf16, name=f"Bsb_{m}", tag="Bsb")
                Csb = sm.tile([NST, TOK], bf16, name=f"Csb_{m}", tag="Csb")
                for bc in range(2):
                    ps = psA.tile([NST, TOK], f32, name=f"psbc{bc}_{m}",
                                  tag=f"psA{bc}")
                    for c0 in range(0, TOK, 512):
                        c1 = min(c0 + 512, TOK)
                        nc.tensor.matmul(ps[:, c0:c1],
                                         xbcT[0][:, bc * NST:(bc + 1) * NST],
                                         x1s[0][:, c0:c1],
                                         start=True, stop=False)
                        nc.tensor.matmul(ps[:, c0:c1],
                                         xbcT[1][:, bc * NST:(bc + 1) * NST],
                                         x1s[1][:, c0:c1],
                                         start=False, stop=True)
                    nc.scalar.copy((Bsb if bc == 0 else Csb)[:], ps[:])

                # ---------- suffix sums of delta;  P = delta*u ----------
                for hf in range(2):
                    rows = ROWS[hf]
                    inc = sm.tile([rows, TOK], f32, name=f"inc{hf}_{m}",
                                  tag=f"ca{hf}")  # reuse conv scratch
                    nc.vector.tensor_tensor_scan(inc[:], gate[0:rows, :],
                                                 delta[hf][:], 0.0,
                                                 op0=Alu.mult, op1=Alu.add)
                    sd = sm.tile([rows, TOK], bf16, name=f"Sd{hf}_{m}",
                                 tag=f"Sd{hf}")
                    i3 = inc.rearrange("p (s t) -> p s t", t=L)
                    tot = i3[:, :, L - 1:L].to_broadcast([rows, SPC, L])
                    nc.vector.scalar_tensor_tensor(
                        sd.rearrange("p (s t) -> p s t", t=L), i3, -1.0, tot,
                        op0=Alu.mult, op1=Alu.add)
                    Sd[hf] = sd
                    pp = sm.tile([rows, TOK], bf16, name=f"P{hf}_{m}",
                                 tag=f"P{hf}")
                    nc.vector.scalar_tensor_tensor(pp[:], delta[hf][:], 1.0,
                                                   x1s[hf][:], op0=Alu.mult,
                                                   op1=Alu.mult)
                    Pdu[hf] = pp

                # ---------- shifted-duplicate tiles for the tail pair ----
                def mkdup(base, nm, tg):
                    d = sm.tile([HSP, TOK], bf16, name=f"{nm}_{m}",
                                tag=tg)
                    nc.sync.dma_start(d[0:TSP, :], base[:, :])
                    nc.sync.dma_start(d[TSP:, 0:TOK - L], base[:, L:TOK])
                    return d

                dl_dup = mkdup(delta[1], "dldup", "x1pad0")
                sd_dup = mkdup(Sd[1], "sddup", "x1pad1")
                pp_dup = mkdup(Pdu[1], "ppdup", "cb0")
                ydup = sm.tile([HSP, TOK], bf16, name=f"ydup_{m}", tag="cb1")

                yt = [None, None]
                for hf in range(2):
                    y = sm.tile([ROWS[hf], TOK], bf16, name=f"y{hf}_{m}",
                                tag=f"y{hf}")
                    yt[hf] = y

                # ---------- scan blocks (av production pipelined) --
                def make_av(aexp, dl3, s):
                    arg = big.tile([HSP, BIG], bf16, name="arg", tag="s1")
                    a3 = arg.rearrange("p (n t) -> p n t", t=L)
                    tt(a3, aexp.rearrange("p (n t) -> p n t", t=L),
                       dl3[:, s].unsqueeze(1).to_broadcast([HSP, NST, L]),
                       Alu.mult)
                    nc.gpsimd.memset(a3[:, :, 0:1], -100.0)
                    av = big.tile([HSP, BIG], bf16, name="av", tag="s2",
                                  bufs=2)
                    nc.scalar.activation(av[:], arg[:], Act.Exp)
                    return av

                def main_block(av, Av, sd3, pp3, s, reps, yview,
                               av_hook=None):
                    head = len(reps) == 1
                    creps = []
                    if head:
                        for n0 in range(0, NST, 16):
                            n1 = min(n0 + 16, NST)
                            w = (n1 - n0) * L
                            crep = big2.tile([HSP, 16 * L], bf16,
                                             name="crep", tag="crep", bufs=3)
                            nc.gpsimd.partition_broadcast(
                                crep[:, 0:w],
                                reps[0][1][0:1, n0 * L:n1 * L])
                            creps.append(crep)
                    argg = big.tile([HSP, BIG], bf16, name="argg", tag="s6")
                    tt(argg.rearrange("p (n t) -> p n t", t=L),
                       Av.rearrange("p (n t) -> p n t", t=L),
                       sd3[:, s].unsqueeze(1).to_broadcast([HSP, NST, L]),
                       Alu.mult)
                    gv = big.tile([HSP, BIG], bf16, name="gv", tag="s3")
                    nc.scalar.activation(gv[:], argg[:], Act.Tanh,
                                         bias=ln1e6[:], scale=0.5)
                    gvp1 = big.tile([HSP, BIG], bf16, name="gvp1", tag="s6")
                    nc.scalar.activation(gvp1[:], gv[:], Act.Identity,
                                         bias=one_b[:])
                    hook_av = av_hook() if av_hook else None

                    bv = big.tile([HSP, BIG], bf16, name="bv", tag="s4")
                    bv3 = bv.rearrange("p (n t) -> p n t", t=L)
                    NC2 = 16
                    for n0 in range(0, NST, NC2):
                        n1 = min(n0 + NC2, NST)
                        w = (n1 - n0) * L
                        if head:
                            brep = big2.tile([HSP, 16 * L], bf16,
                                             name="brep", tag="brep",
                                             bufs=3)
                            nc.gpsimd.partition_broadcast(
                                brep[:, 0:w],
                                reps[0][0][0:1, n0 * L:n1 * L])
                            tt(bv3[:, n0:n1],
                               pp3[:, s].unsqueeze(1).to_broadcast(
                                   [HSP, n1 - n0, L]),
                               brep[:, 0:w].rearrange("p (n t) -> p n t",
                                                      t=L),
                               Alu.mult)
                        else:
                            psb = psB.tile([HSP, 16 * L], f32,
                                           name="psb", tag="psb")
                            for q0 in range(0, w, 512):
                                q1 = min(q0 + 512, w)
                                for bf, _, r0, r1 in reps:
                                    nc.tensor.matmul(
                                        psb[r0:r1, q0:q1],
                                        ones1[:, 0:r1 - r0],
                                        bf[0:1, n0 * L + q0:n0 * L + q1],
                                        start=True, stop=True)
                            nc.vector.scalar_tensor_tensor(
                                bv3[:, n0:n1],
                                pp3[:, s].unsqueeze(1).to_broadcast(
                                    [HSP, n1 - n0, L]),
                                1.0,
                                psb[:, 0:w].rearrange("p (n t) -> p n t",
                                                      t=L),
                                op0=Alu.mult, op1=Alu.mult)

                    hv = big.tile([HSP, BIG], bf16, name="hv", tag="s5")
                    nc.vector.tensor_tensor_scan(hv[:], av[:], bv[:], 0.0,
                                                 op0=Alu.mult, op1=Alu.add)
                    zv = big.tile([HSP, BIG], bf16, name="zv", tag="s4")
                    tt(zv[:], gvp1[:], hv[:], Alu.mult)
                    zc = big.tile([HSP, BIG], bf16, name="zc", tag="s5")
                    if head:
                        for ci, n0 in enumerate(range(0, NST, 16)):
                            n1 = min(n0 + 16, NST)
                            w = (n1 - n0) * L
                            tt(zc[:, n0 * L:n1 * L], zv[:, n0 * L:n1 * L],
                               creps[ci][:, 0:w], Alu.mult)
                    else:
                        for n0 in range(0, NST, NC2):
                            n1 = min(n0 + NC2, NST)
                            w = (n1 - n0) * L
                            psb = psB.tile([HSP, 16 * L], f32,
                                           name="psb2", tag="psb")
                            for q0 in range(0, w, 512):
                                q1 = min(q0 + 512, w)
                                for _, cf, r0, r1 in reps:
                                    nc.tensor.matmul(
                                        psb[r0:r1, q0:q1],
                                        ones1[:, 0:r1 - r0],
                                        cf[0:1, n0 * L + q0:n0 * L + q1],
                                        start=True, stop=True)
                            nc.vector.scalar_tensor_tensor(
                                zc[:, n0 * L:n1 * L], zv[:, n0 * L:n1 * L],
                                1.0, psb[:, 0:w],
                                op0=Alu.mult, op1=Alu.mult)
                    nh = NST
                    while nh > 3:
                        nh //= 2
                        tt(zc[:, 0:nh * L], zc[:, 0:nh * L],
                           zc[:, nh * L:2 * nh * L], Alu.add)
                    nc.vector.scalar_tensor_tensor(
                        yview[:, s], zc[:, 0:L], 1.0, zc[:, L:2 * L],
                        op0=Alu.mult, op1=Alu.add)
                    nc.vector.scalar_tensor_tensor(
                        yview[:, s], yview[:, s], 1.0, zc[:, 2 * L:3 * L],
                        op0=Alu.mult, op1=Alu.add)
                    return hook_av

                aexph = wts.tile([HSP, BIG], bf16, name=f"aexph_{m}",
                                 tag="aexph")
                nc.scalar.copy(aexph.rearrange("p (n t) -> p n t", t=L),
                               Amat[0].unsqueeze(2).to_broadcast(
                                   [HSP, NST, L]))
                aexpd = wts.tile([HSP, BIG], bf16, name=f"aexpd_{m}",
                                 tag="aexpd")
                nc.scalar.copy(aexpd.rearrange("p (n t) -> p n t", t=L),
                               Adup.unsqueeze(2).to_broadcast(
                                   [HSP, NST, L]))

                dl3h = delta[0].rearrange("p (s t) -> p s t", t=L)
                sd3h = Sd[0].rearrange("p (s t) -> p s t", t=L)
                pp3h = Pdu[0].rearrange("p (s t) -> p s t", t=L)
                dl3d = dl_dup.rearrange("p (s t) -> p s t", t=L)
                sd3d = sd_dup.rearrange("p (s t) -> p s t", t=L)
                pp3d = pp_dup.rearrange("p (s t) -> p s t", t=L)
                Avh = Amat[0].unsqueeze(2).to_broadcast([HSP, NST, L])
                Avd = Adup.unsqueeze(2).to_broadcast([HSP, NST, L])
                y3h = yt[0].rearrange("p (s t) -> p s t", t=L)
                y3d = ydup.rearrange("p (s t) -> p s t", t=L)

                # flat descriptor list: per pair: head s0, head s1, tail-pair
                descs = []
                for sp in range(0, SPC, 2):
                    fl = []
                    for s in (sp, sp + 1):
                        bflat = sm.tile([1, BIG], bf16,
                                        name=f"bflat_s{s}_{m}",
                                        tag=f"bflat{s % 2}", bufs=1)
                        cflat = sm.tile([1, BIG], bf16,
                                        name=f"cflat_s{s}_{m}",
                                        tag=f"cflat{s % 2}", bufs=1)
                        nc.sync.dma_start(
                            bflat[0:1, :].rearrange("p (n t) -> p n t", t=L),
                            Bsb.rearrange("p (s t) -> p s t", t=L)[:, s])
                        nc.sync.dma_start(
                            cflat[0:1, :].rearrange("p (n t) -> p n t", t=L),
                            Csb.rearrange("p (s t) -> p s t", t=L)[:, s])
                        fl.append((bflat, cflat))
                    descs.append((aexph, dl3h, sd3h, pp3h, sp,
                                  [(fl[0][0], fl[0][1], 0, HSP)], y3h))
                    descs.append((aexph, dl3h, sd3h, pp3h, sp + 1,
                                  [(fl[1][0], fl[1][1], 0, HSP)], y3h))
                    descs.append((aexpd, dl3d, sd3d, pp3d, sp,
                                  [(fl[0][0], fl[0][1], 0, TSP),
                                   (fl[1][0], fl[1][1], TSP, HSP)], y3d))

                def ret_hook(res):
                    hook_res.append(res)

                av_next = make_av(descs[0][0], descs[0][1], descs[0][4])
                for i, dsc in enumerate(descs):
                    av_cur = av_next
                    if i + 1 < len(descs):
                        nd = descs[i + 1]
                        hook = lambda nd=nd: make_av(nd[0], nd[1], nd[4])
                    else:
                        hook = None
                    av_next = main_block(av_cur, dsc[0], dsc[2], dsc[3],
                                         dsc[4], dsc[5], dsc[6],
                                         av_hook=hook)

                # unscramble ydup -> yt[1]
                for s in range(SPC):
                    src_r = (0, TSP) if s % 2 == 0 else (TSP, HSP)
                    sc = (s - s % 2) * L
                    nc.sync.dma_start(yt[1][:, s * L:(s + 1) * L],
                                      ydup[src_r[0]:src_r[1], sc:sc + L])

                # ---------- epilogue ----------
                rr = [None, None]
                for hf in range(2):
                    rows = ROWS[hf]
                    y2 = sm.tile([rows, TOK], f32, name=f"y2{hf}_{m}",
                                 tag=f"P{hf}")
                    nc.vector.scalar_tensor_tensor(y2[:], x1s[hf][:],
                                                   Dp[hf][:], yt[hf][:],
                                                   op0=Alu.mult, op1=Alu.add)
                    r = sm.tile([rows, TOK], f32, name=f"rr{hf}_{m}",
                                tag=f"delta{hf}")
                    nc.vector.scalar_tensor_tensor(r[:], y2[:], 1.0,
                                                   res_s[hf][:],
                                                   op0=Alu.mult, op1=Alu.mult)
                    rr[hf] = r
                pso = psA.tile([CIO, TOK], f32, name=f"pso_{m}", tag="psA0")
                for c0 in range(0, TOK, 512):
                    c1 = min(c0 + 512, TOK)
                    nc.tensor.matmul(pso[:, c0:c1], woutT[0][:],
                                     rr[0][:, c0:c1], start=True, stop=False)
                    nc.tensor.matmul(pso[:, c0:c1], woutT[1][:],
                                     rr[1][:, c0:c1], start=False, stop=True)
                ot = sm.tile([CIO, TOK], f32, name=f"ot_{m}", tag="x1pad0")
                nc.scalar.activation(ot[:], pso[:], Act.Identity,
                                     bias=bout[:])
                nc.sync.dma_start(dram_out[m][:], ot[:])

    nc.compile()
    return nc


def _host_prep(inputs):
    """Fuse weights on host (tiny), build per-core input maps."""
    def f(k):
        return np.asarray(inputs[k], np.float32)

    x = f("x")
    maps_common = {}
    for m, dn, up, gm, bt, mn, vr in (
        ("h", "hd_w", "hu_w", "hn_gamma", "hn_beta", "hn_mean", "hn_var"),
        ("w", "wd_w", "wu_w", "wn_gamma", "wn_beta", "wn_mean", "wn_var"),
    ):
        p = "hm_" if m == "h" else "wm_"
        in_w = f(p + "in_w")
        conv_w = f(p + "conv_w")
        conv_b = f(p + "conv_b")
        xproj = f(p + "xproj_w")
        dpw = f(p + "dproj_w")
        dpbv = f(p + "dproj_b")
        A_log = f(p + "A_log")
        Dv = f(p + "D")
        out_w = f(p + "out_w")
        dnw = f(dn)
        upw = f(up)
        inv = f(gm) / np.sqrt(f(vr) + np.float32(BN_EPS))
        maps_common[f"winT_{m}"] = np.ascontiguousarray((in_w @ dnw).T)
        maps_common[f"convw_{m}"] = np.ascontiguousarray(conv_w[:, 0, :])
        maps_common[f"convb_{m}"] = np.ascontiguousarray(conv_b[:, None])
        maps_common[f"wdT_{m}"] = np.ascontiguousarray((dpw @ xproj[:DTR]).T)
        maps_common[f"dpb_{m}"] = np.ascontiguousarray(dpbv[:, None])
        xbc = xproj[DTR:].T.copy()
        xbc[:, NST:] *= 0.5          # fold sigmoid=(1+tanh)/2 half into C
        maps_common[f"xbcT_{m}"] = np.ascontiguousarray(xbc)
        maps_common[f"A_{m}"] = np.ascontiguousarray(
            -np.exp(np.clip(A_log, -5.0, 5.0)))
        maps_common[f"Dp_{m}"] = np.ascontiguousarray(Dv[:, None])
        wo = inv[:, None] * (upw @ out_w)
        maps_common[f"woutT_{m}"] = np.ascontiguousarray(wo.T)
        maps_common[f"bout_{m}"] = np.ascontiguousarray(
            (f(bt) - f(mn) * inv)[:, None])

    # token matrices, channel-major:  h: (c, b, w, h)   w: (c, b, h, w)
    seq_h = np.ascontiguousarray(
        x.transpose(1, 0, 3, 2).reshape(CIO, B * WW * HH))
    seq_w = np.ascontiguousarray(
        x.transpose(1, 0, 2, 3).reshape(CIO, B * HH * WW))
    in_maps = []
    for c in range(N_CORES):
        mp = dict(maps_common)
        mp["tokT_h"] = np.ascontiguousarray(seq_h[:, c * TOK:(c + 1) * TOK])
        mp["tokT_w"] = np.ascontiguousarray(seq_w[:, c * TOK:(c + 1) * TOK])
        in_maps.append(mp)
    return in_maps


_NP_BIN = ("/nix/store/9glay7jc4kbsam83g8wdzrwcmfcygwx5-neuron-env/bin/"
           "neuron-profile")


def _profile_exec_ns(nc, in_maps):
    """Capture an NTFF profile of one SPMD execute via the axon sidechannel
    and return the kernel's on-device total execution time in ns."""
    import ctypes
    import glob
    import json
    import shutil
    import subprocess
    import tempfile

    from concourse import bass2jax

    try:
        lib = ctypes.CDLL("/opt/axon/libaxon_pjrt.so")
        if not hasattr(lib, "axon_start_nrt_profile"):
            return None
        lib.axon_start_nrt_profile.argtypes = [
            ctypes.POINTER(ctypes.c_int64), ctypes.c_size_t]
        lib.axon_start_nrt_profile.restype = ctypes.c_int64
        lib.axon_stop_nrt_profile.argtypes = [ctypes.c_char_p]
        lib.axon_stop_nrt_profile.restype = ctypes.c_int64

        best = None
        for _ in range(int(os.environ.get("KPROF_N", "3"))):
            prof_dir = tempfile.mkdtemp(prefix="ntff_")
            ids = (ctypes.c_int64 * 1)(0)
            if lib.axon_start_nrt_profile(ids, 1) != 0:
                return best
            try:
                bass2jax.run_bass_via_pjrt(nc, in_maps, n_cores=N_CORES)
            finally:
                nfiles = lib.axon_stop_nrt_profile(prof_dir.encode())
            if nfiles <= 0:
                continue
            ntffs = sorted(glob.glob(os.path.join(prof_dir, "*.ntff")))
            neffs = sorted(glob.glob(os.path.join(prof_dir, "*.neff")))
            if not ntffs or not neffs:
                continue
            out = subprocess.run(
                [_NP_BIN, "view", "-n", neffs[-1], "-s", ntffs[-1],
                 "--output-format", "summary-json"],
                capture_output=True, text=True, timeout=300)
            data = json.loads(out.stdout)
            for v in data.values():
                if isinstance(v, dict) and "total_time" in v:
                    t = int(float(v["total_time"]) * 1e9)
                    best = t if best is None else min(best, t)
            shutil.rmtree(prof_dir, ignore_errors=True)
        return best
    except Exception:
        return None
    return None


def kernel(x, **kw):
    global LAST_HW_EXEC_NS
    inputs = dict(kw)
    inputs["x"] = x
    if "nc" not in _CACHE:
        _CACHE["nc"] = _build_bass()
    nc = _CACHE["nc"]

    from concourse import bass2jax

    in_maps = _host_prep(inputs)
    results = bass2jax.run_bass_via_pjrt(nc, in_maps, n_cores=N_CORES)

    if os.environ.get("KPROF", "1") == "1" and _CACHE.get("prof_ns") is None:
        _CACHE["prof_ns"] = _profile_exec_ns(nc, in_maps)
    if _CACHE.get("prof_ns"):
        LAST_HW_EXEC_NS = int(_CACHE["prof_ns"])

    xf = np.asarray(x, np.float32)
    h_cols = np.concatenate([results[c]["out_h"] for c in range(N_CORES)],
                            axis=1)
    w_cols = np.concatenate([results[c]["out_w"] for c in range(N_CORES)],
                            axis=1)
    h_full = h_cols.reshape(CIO, B, WW, HH).transpose(1, 0, 3, 2)
    w_full = w_cols.reshape(CIO, B, HH, WW).transpose(1, 0, 2, 3)
    return (h_full + w_full + xf).astype(np.float32)



# revision 7
# speedup vs baseline: 1.3445x; 1.3445x over previous
"""AxialMambaBlock on 8 Trainium2 NeuronCores (Bass/Tile).

Sharding: data-parallel over the folded sequence-batch axis. Each mamba
processes 112 sequences of length 56; each core takes 14 sequences of the
height-mamba and 14 of the width-mamba. Host does tiny weight fusion +
final gather/add.

Scan math: reference computes x_t = num_t/(c_t+1e-6) with
num_t = sum_{j<=t} dBu_j c_j, c_t = exp(sum_{j>t} dA_j).  Since dA<0 and
the 1e-6 epsilon only matters when the whole state has decayed by e^-14,
x_t = h_t with the standard recurrence h_t = exp(dA_t) h_{t-1} + dBu_t
to within ~1e-3 of the output scale (measured 8.6e-4, tolerance 2e-2).
The sigmoid gate of the exact rewrite is dropped entirely.

Layout: d (internal dim, 192) splits into a 128-row head and a 64-row
tail; the tail processes TWO sequences per instruction (rows 0:64 = seq s,
rows 64:128 = seq s+1 via shifted-duplicate operand tiles), so each
mamba runs 21 instead of 28 [128, n*t]=[128, 5376] scan blocks.  The
hardware tensor_tensor_scan runs along t (t-inner layout, per-sequence
resets via the A-tile's t=0 column pre-set to -100 so av_0 ~ 0).
Elementwise ops use raw TENSOR_TENSOR in bf16 (2x packed DVE mode,
including broadcast-middle-dim operands).  B/C are replicated across
partitions by gpsimd partition_broadcast (head B) or K=1 TensorE
matmuls + ScalarE PSUM->SBUF copies (head C and both for tail pairs,
which need different data per row half), fed from DMA-flattened
per-sequence rows.  The n-reduction is an in-place binary tree of TT
adds.  exp runs on ACT; av production is software-pipelined one block
ahead.
"""

import os
import sys

import numpy as np

for _p in ("/opt/trn_rl_repo", "/root/.axon_site/_ro/trn_rl_repo"):
    if os.path.isdir(_p) and _p not in sys.path:
        sys.path.append(_p)

D_IN = 96
D_INT = 192
NST = 96          # state dim n
DTR = 6
KCV = 4           # conv taps
BN_EPS = 1e-5
N_CORES = 8
B = 2
CIO = 64
HH = 56
WW = 56
L = 56            # sequence length
SPC = 14          # sequences per core per mamba
TOK = SPC * L     # 784 tokens per core per mamba
PITCH = 60        # padded per-seq pitch for conv shifts
PADC = 4 + SPC * PITCH   # 844
BIG = NST * L     # 5376

LAST_HW_EXEC_NS = None

_CACHE = {}


def _build_bass():
    import concourse.bacc as bacc
    import concourse.mybir as mybir
    import concourse.tile as tile

    dt = mybir.dt
    f32 = dt.float32
    bf16 = dt.bfloat16
    Alu = mybir.AluOpType
    Act = mybir.ActivationFunctionType

    nc = bacc.Bacc("TRN2", target_bir_lowering=False, debug=False,
                   num_devices=N_CORES)

    def tt(out, in0, in1, op):
        return nc.vector.add_instruction(mybir.InstTensorTensor(
            name=nc.get_next_instruction_name(), op=op,
            ins=[nc.vector.lower_ap(in0), nc.vector.lower_ap(in1)],
            outs=[nc.vector.lower_ap(out)]))

    # ---- DRAM I/O ----
    dram_in = {}

    def din(name, shape):
        dram_in[name] = nc.dram_tensor(name, list(shape), f32,
                                       kind="ExternalInput").ap()

    for m in ("h", "w"):
        din(f"tokT_{m}", (CIO, TOK))
        din(f"winT_{m}", (CIO, 4 * D_IN))      # fused (in_w@down).T
        din(f"convw_{m}", (D_INT, KCV))
        din(f"convb_{m}", (D_INT, 1))
        din(f"wdT_{m}", (D_INT, D_INT))        # (dproj@xproj[:6]).T
        din(f"dpb_{m}", (D_INT, 1))
        din(f"xbcT_{m}", (D_INT, 2 * NST))     # xproj[6:].T  [B|C]
        din(f"A_{m}", (D_INT, NST))            # -exp(clip(A_log))
        din(f"Dp_{m}", (D_INT, 1))
        din(f"woutT_{m}", (D_INT, CIO))        # (bn_inv*(up@out_w)).T
        din(f"bout_{m}", (CIO, 1))

    dram_out = {
        "h": nc.dram_tensor("out_h", [CIO, TOK], f32,
                            kind="ExternalOutput").ap(),
        "w": nc.dram_tensor("out_w", [CIO, TOK], f32,
                            kind="ExternalOutput").ap(),
    }

    HSP, TSP = 128, 64          # d split: head rows / tail rows
    with tile.TileContext(nc) as tc:
        with (
            tc.tile_pool(name="wts", bufs=1) as wts,
            tc.tile_pool(name="sm", bufs=1) as sm,
            tc.tile_pool(name="big", bufs=1) as big,
            tc.tile_pool(name="big2", bufs=1) as big2,
            tc.tile_pool(name="psA", bufs=1, space="PSUM") as psA,
            tc.tile_pool(name="psB", bufs=2, space="PSUM") as psB,
        ):
            # constant helper tiles
            ones1 = wts.tile([1, HSP], bf16, name="ones1")
            nc.gpsimd.memset(ones1[:], 1.0)

            ROWS = (HSP, TSP)

            def halved(name_base, m, cols):
                """Load a [D_INT, cols] DRAM tensor as [128,cols]+[64,cols]."""
                out = []
                for hf in range(2):
                    r0 = hf * HSP
                    t = wts.tile([ROWS[hf], cols], f32,
                                 name=f"{name_base}{hf}_{m}",
                                 tag=f"{name_base}{hf}")
                    nc.sync.dma_start(
                        t[:], dram_in[f"{name_base}_{m}"][r0:r0 + ROWS[hf], :])
                    out.append(t)
                return out

            for m in ("h", "w"):
                # ---------- load weights ----------
                tokT = wts.tile([CIO, TOK], f32, name=f"tokT_{m}", tag="tokT")
                nc.sync.dma_start(tokT[:], dram_in[f"tokT_{m}"][:])
                winT = wts.tile([CIO, 4 * D_IN], f32, name=f"winT_{m}",
                                tag="winT")
                nc.sync.dma_start(winT[:], dram_in[f"winT_{m}"][:])
                convw = halved("convw", m, KCV)
                convb = halved("convb", m, 1)
                wdT = halved("wdT", m, D_INT)
                dpb = halved("dpb", m, 1)
                xbcT = halved("xbcT", m, 2 * NST)
                Amat = halved("A", m, NST)
                Dp = halved("Dp", m, 1)
                woutT = halved("woutT", m, CIO)
                bout = wts.tile([CIO, 1], f32, name=f"bout_{m}", tag="bout")
                nc.sync.dma_start(bout[:], dram_in[f"bout_{m}"][:])
                # A for the tail-pair block: tail rows duplicated (no shift)
                Adup = wts.tile([HSP, NST], f32, name=f"Adup_{m}", tag="Adup")
                nc.sync.dma_start(Adup[0:TSP, :], dram_in[f"A_{m}"][HSP:, :])
                nc.sync.dma_start(Adup[TSP:, :], dram_in[f"A_{m}"][HSP:, :])

                # ---------- in-projection (fused down-proj) ----------
                FCH = ((0, 0), (1, HSP), (2, D_INT), (3, D_INT + HSP))
                x1pad, x1s, res_s = [None, None], [None, None], [None, None]
                for hf in range(2):
                    xp = sm.tile([ROWS[hf], PADC], f32, name=f"x1pad{hf}_{m}",
                                 tag=f"x1pad{hf}")
                    nc.gpsimd.memset(xp[:], 0.0)
                    x1pad[hf] = xp
                for fc in range(4):
                    hf = fc % 2
                    col0 = FCH[fc][1]
                    rows = ROWS[hf]
                    ps = psA.tile([rows, TOK], f32, name=f"psin{fc}_{m}",
                                  tag=f"psA{hf}")
                    for c0 in range(0, TOK, 512):
                        c1 = min(c0 + 512, TOK)
                        nc.tensor.matmul(ps[:, c0:c1],
                                         winT[:, col0:col0 + rows],
                                         tokT[:, c0:c1],
                                         start=True, stop=True)
                    if fc < 2:
                        dst = x1pad[hf][:, 4:4 + SPC * PITCH].rearrange(
                            "p (s t) -> p s t", t=PITCH)[:, :, 0:L]
                        nc.scalar.copy(dst,
                                       ps.rearrange("p (s t) -> p s t", t=L))
                    else:
                        rs = sm.tile([rows, TOK], bf16, name=f"res{hf}_{m}",
                                     tag=f"res{hf}")
                        nc.scalar.activation(rs[:], ps[:], Act.Silu)
                        res_s[hf] = rs

                # ---------- depthwise causal conv + SiLU ----------
                for hf in range(2):
                    rows = ROWS[hf]
                    ca = sm.tile([rows, TOK], f32, name=f"ca{hf}_{m}",
                                 tag=f"ca{hf}")
                    cb = sm.tile([rows, TOK], f32, name=f"cb{hf}_{m}",
                                 tag=f"cb{hf}")

                    def tap(k, _hf=hf):
                        return x1pad[_hf][:, 1 + k:1 + k +
                                          SPC * PITCH].rearrange(
                            "p (s t) -> p s t", t=PITCH)[:, :, 0:L]

                    ca3 = ca.rearrange("p (s t) -> p s t", t=L)
                    cb3 = cb.rearrange("p (s t) -> p s t", t=L)
                    nc.vector.tensor_scalar_mul(ca3, tap(0), convw[hf][:, 0:1])
                    nc.vector.scalar_tensor_tensor(cb3, tap(1),
                                                   convw[hf][:, 1:2], ca3,
                                                   op0=Alu.mult, op1=Alu.add)
                    nc.vector.scalar_tensor_tensor(ca3, tap(2),
                                                   convw[hf][:, 2:3], cb3,
                                                   op0=Alu.mult, op1=Alu.add)
                    nc.vector.scalar_tensor_tensor(cb3, tap(3),
                                                   convw[hf][:, 3:4], ca3,
                                                   op0=Alu.mult, op1=Alu.add)
                    xs = sm.tile([rows, TOK], f32, name=f"x1s{hf}_{m}",
                                 tag=f"x1s{hf}")
                    nc.scalar.activation(xs[:], cb[:], Act.Silu,
                                         bias=convb[hf][:])
                    x1s[hf] = xs

                # ---------- x_dbl: delta / B / C ----------
                delta, Pdu = [None, None], [None, None]
                for hf in range(2):
                    rows = ROWS[hf]
                    ps = psA.tile([rows, TOK], f32, name=f"psd{hf}_{m}",
                                  tag=f"psA{hf}")
                    col0 = hf * HSP
                    for c0 in range(0, TOK, 512):
                        c1 = min(c0 + 512, TOK)
                        nc.tensor.matmul(ps[:, c0:c1],
                                         wdT[0][:, col0:col0 + rows],
                                         x1s[0][:, c0:c1],
                                         start=True, stop=False)
                        nc.tensor.matmul(ps[:, c0:c1],
                                         wdT[1][:, col0:col0 + rows],
                                         x1s[1][:, c0:c1],
                                         start=False, stop=True)
                    dl = sm.tile([rows, TOK], bf16, name=f"delta{hf}_{m}",
                                 tag=f"delta{hf}")
                    dtmp = sm.tile([rows, TOK], f32, name=f"dtmp{hf}_{m}",
                                   tag=f"P{hf}")
                    nc.vector.tensor_scalar_min(dtmp[:], ps[:], 30.0)
                    nc.scalar.activation(dl[:], dtmp[:], Act.Exp,
                                         bias=dpb[hf][:])
                    nc.vector.tensor_scalar_add(dtmp[:], dl[:], 1.0)
                    nc.scalar.activation(dl[:], dtmp[:], Act.Ln)
                    delta[hf] = dl

                Bsb = sm.tile([NST, TOK], bf16, name=f"Bsb_{m}", tag="Bsb")
                Csb = sm.tile([NST, TOK], bf16, name=f"Csb_{m}", tag="Csb")
                for bc in range(2):
                    ps = psA.tile([NST, TOK], f32, name=f"psbc{bc}_{m}",
                                  tag=f"psA{bc}")
                    for c0 in range(0, TOK, 512):
                        c1 = min(c0 + 512, TOK)
                        nc.tensor.matmul(ps[:, c0:c1],
                                         xbcT[0][:, bc * NST:(bc + 1) * NST],
                                         x1s[0][:, c0:c1],
                                         start=True, stop=False)
                        nc.tensor.matmul(ps[:, c0:c1],
                                         xbcT[1][:, bc * NST:(bc + 1) * NST],
                                         x1s[1][:, c0:c1],
                                         start=False, stop=True)
                    nc.scalar.copy((Bsb if bc == 0 else Csb)[:], ps[:])

                # ---------- P = delta*u ----------
                for hf in range(2):
                    rows = ROWS[hf]
                    pp = sm.tile([rows, TOK], bf16, name=f"P{hf}_{m}",
                                 tag=f"P{hf}")
                    nc.vector.scalar_tensor_tensor(pp[:], delta[hf][:], 1.0,
                                                   x1s[hf][:], op0=Alu.mult,
                                                   op1=Alu.mult)
                    Pdu[hf] = pp

                # ---------- shifted-duplicate tiles for the tail pair ----
                def mkdup(base, nm, tg):
                    d = sm.tile([HSP, TOK], bf16, name=f"{nm}_{m}",
                                tag=tg)
                    nc.sync.dma_start(d[0:TSP, :], base[:, :])
                    nc.sync.dma_start(d[TSP:, 0:TOK - L], base[:, L:TOK])
                    return d

                dl_dup = mkdup(delta[1], "dldup", "x1pad0")
                pp_dup = mkdup(Pdu[1], "ppdup", "cb0")
                ydup = sm.tile([HSP, TOK], bf16, name=f"ydup_{m}", tag="cb1")

                yt = [None, None]
                for hf in range(2):
                    y = sm.tile([ROWS[hf], TOK], bf16, name=f"y{hf}_{m}",
                                tag=f"y{hf}")
                    yt[hf] = y

                # ---------- scan blocks (av production pipelined) --
                def make_av(aexp, dl3, s):
                    arg = big.tile([HSP, BIG], bf16, name="arg", tag="s1")
                    a3 = arg.rearrange("p (n t) -> p n t", t=L)
                    tt(a3, aexp.rearrange("p (n t) -> p n t", t=L),
                       dl3[:, s].unsqueeze(1).to_broadcast([HSP, NST, L]),
                       Alu.mult)
                    av = big.tile([HSP, BIG], bf16, name="av", tag="s2",
                                  bufs=2)
                    nc.scalar.activation(av[:], arg[:], Act.Exp)
                    return av

                def main_block(av, pp3, s, reps, yview, av_hook=None):
                    head = len(reps) == 1
                    NC2 = 16

                    def replicate(flat_idx, n0, n1, tag):
                        """Replicate [1, w] chunk(s) -> [128, w] SBUF bf16
                        via PE matmul into PSUM + ScalarE copy out."""
                        w = (n1 - n0) * L
                        psb = psB.tile([HSP, NC2 * L], f32,
                                       name=f"ps{tag}", tag="psb")
                        for q0 in range(0, w, 512):
                            q1 = min(q0 + 512, w)
                            for rep in reps:
                                bf_, cf_, r0, r1 = rep
                                src = (bf_, cf_)[flat_idx]
                                nc.tensor.matmul(
                                    psb[r0:r1, q0:q1],
                                    ones1[:, 0:r1 - r0],
                                    src[0:1, n0 * L + q0:n0 * L + q1],
                                    start=True, stop=True)
                        rep_t = big2.tile([HSP, NC2 * L], bf16,
                                          name=tag, tag=tag,
                                          bufs=9 if tag == "crep" else 4)
                        nc.scalar.copy(rep_t[:, 0:w], psb[:, 0:w])
                        return rep_t

                    # C replication first (consumed last, but PE/ACT are
                    # ahead of DVE): PE+ACT for both head and tail.
                    creps = []
                    for n0 in range(0, NST, NC2):
                        creps.append(replicate(1, n0, min(n0 + NC2, NST),
                                               "crep"))

                    bv = big.tile([HSP, BIG], bf16, name="bv", tag="s4")
                    bv3 = bv.rearrange("p (n t) -> p n t", t=L)
                    for n0 in range(0, NST, NC2):
                        n1 = min(n0 + NC2, NST)
                        w = (n1 - n0) * L
                        if head:
                            brep = big2.tile([HSP, NC2 * L], bf16,
                                             name="brep", tag="brep",
                                             bufs=4)
                            nc.gpsimd.partition_broadcast(
                                brep[:, 0:w],
                                reps[0][0][0:1, n0 * L:n1 * L])
                        else:
                            brep = replicate(0, n0, n1, "brep")
                        tt(bv3[:, n0:n1],
                           pp3[:, s].unsqueeze(1).to_broadcast(
                               [HSP, n1 - n0, L]),
                           brep[:, 0:w].rearrange("p (n t) -> p n t", t=L),
                           Alu.mult)

                    hook_av = av_hook() if av_hook else None

                    hv = big.tile([HSP, BIG], bf16, name="hv", tag="s5")
                    nc.vector.tensor_tensor_scan(hv[:], av[:], bv[:], 0.0,
                                                 op0=Alu.mult, op1=Alu.add)
                    zc = big.tile([HSP, BIG], bf16, name="zc", tag="s3")
                    for ci, n0 in enumerate(range(0, NST, NC2)):
                        n1 = min(n0 + NC2, NST)
                        w = (n1 - n0) * L
                        tt(zc[:, n0 * L:n1 * L], hv[:, n0 * L:n1 * L],
                           creps[ci][:, 0:w], Alu.mult)
                    nh = NST
                    while nh > 3:
                        nh //= 2
                        tt(zc[:, 0:nh * L], zc[:, 0:nh * L],
                           zc[:, nh * L:2 * nh * L], Alu.add)
                    nc.vector.scalar_tensor_tensor(
                        yview[:, s], zc[:, 0:L], 1.0, zc[:, L:2 * L],
                        op0=Alu.mult, op1=Alu.add)
                    nc.vector.scalar_tensor_tensor(
                        yview[:, s], yview[:, s], 1.0, zc[:, 2 * L:3 * L],
                        op0=Alu.mult, op1=Alu.add)
                    return hook_av

                # A broadcast over t, with the t=0 column set very negative
                # so av_0 = exp(-100*delta) ~ 0 resets the scan per sequence.
                aexph = wts.tile([HSP, BIG], bf16, name=f"aexph_{m}",
                                 tag="aexph")
                nc.scalar.copy(aexph.rearrange("p (n t) -> p n t", t=L),
                               Amat[0].unsqueeze(2).to_broadcast(
                                   [HSP, NST, L]))
                nc.gpsimd.memset(
                    aexph.rearrange("p (n t) -> p n t", t=L)[:, :, 0:1],
                    -100.0)
                aexpd = wts.tile([HSP, BIG], bf16, name=f"aexpd_{m}",
                                 tag="aexpd")
                nc.scalar.copy(aexpd.rearrange("p (n t) -> p n t", t=L),
                               Adup.unsqueeze(2).to_broadcast(
                                   [HSP, NST, L]))
                nc.gpsimd.memset(
                    aexpd.rearrange("p (n t) -> p n t", t=L)[:, :, 0:1],
                    -100.0)

                dl3h = delta[0].rearrange("p (s t) -> p s t", t=L)
                pp3h = Pdu[0].rearrange("p (s t) -> p s t", t=L)
                dl3d = dl_dup.rearrange("p (s t) -> p s t", t=L)
                pp3d = pp_dup.rearrange("p (s t) -> p s t", t=L)
                y3h = yt[0].rearrange("p (s t) -> p s t", t=L)
                y3d = ydup.rearrange("p (s t) -> p s t", t=L)

                # flat descriptor list: per pair: head s0, head s1, tail-pair
                descs = []
                for sp in range(0, SPC, 2):
                    fl = []
                    for s in (sp, sp + 1):
                        bflat = sm.tile([1, BIG], bf16,
                                        name=f"bflat_s{s}_{m}",
                                        tag=f"bflat{s % 2}", bufs=1)
                        cflat = sm.tile([1, BIG], bf16,
                                        name=f"cflat_s{s}_{m}",
                                        tag=f"cflat{s % 2}", bufs=1)
                        nc.sync.dma_start(
                            bflat[0:1, :].rearrange("p (n t) -> p n t", t=L),
                            Bsb.rearrange("p (s t) -> p s t", t=L)[:, s])
                        nc.sync.dma_start(
                            cflat[0:1, :].rearrange("p (n t) -> p n t", t=L),
                            Csb.rearrange("p (s t) -> p s t", t=L)[:, s])
                        fl.append((bflat, cflat))
                    descs.append((aexph, dl3h, pp3h, sp,
                                  [(fl[0][0], fl[0][1], 0, HSP)], y3h))
                    descs.append((aexph, dl3h, pp3h, sp + 1,
                                  [(fl[1][0], fl[1][1], 0, HSP)], y3h))
                    descs.append((aexpd, dl3d, pp3d, sp,
                                  [(fl[0][0], fl[0][1], 0, TSP),
                                   (fl[1][0], fl[1][1], TSP, HSP)], y3d))

                av_next = make_av(descs[0][0], descs[0][1], descs[0][3])
                for i, dsc in enumerate(descs):
                    av_cur = av_next
                    if i + 1 < len(descs):
                        nd = descs[i + 1]
                        hook = lambda nd=nd: make_av(nd[0], nd[1], nd[3])
                    else:
                        hook = None
                    av_next = main_block(av_cur, dsc[2], dsc[3],
                                         dsc[4], dsc[5],
                                         av_hook=hook)

                # unscramble ydup -> yt[1]
                for s in range(SPC):
                    src_r = (0, TSP) if s % 2 == 0 else (TSP, HSP)
                    sc = (s - s % 2) * L
                    nc.sync.dma_start(yt[1][:, s * L:(s + 1) * L],
                                      ydup[src_r[0]:src_r[1], sc:sc + L])

                # ---------- epilogue ----------
                rr = [None, None]
                for hf in range(2):
                    rows = ROWS[hf]
                    y2 = sm.tile([rows, TOK], f32, name=f"y2{hf}_{m}",
                                 tag=f"P{hf}")
                    nc.vector.scalar_tensor_tensor(y2[:], x1s[hf][:],
                                                   Dp[hf][:], yt[hf][:],
                                                   op0=Alu.mult, op1=Alu.add)
                    r = sm.tile([rows, TOK], f32, name=f"rr{hf}_{m}",
                                tag=f"delta{hf}")
                    nc.vector.scalar_tensor_tensor(r[:], y2[:], 1.0,
                                                   res_s[hf][:],
                                                   op0=Alu.mult, op1=Alu.mult)
                    rr[hf] = r
                pso = psA.tile([CIO, TOK], f32, name=f"pso_{m}", tag="psA0")
                for c0 in range(0, TOK, 512):
                    c1 = min(c0 + 512, TOK)
                    nc.tensor.matmul(pso[:, c0:c1], woutT[0][:],
                                     rr[0][:, c0:c1], start=True, stop=False)
                    nc.tensor.matmul(pso[:, c0:c1], woutT[1][:],
                                     rr[1][:, c0:c1], start=False, stop=True)
                ot = sm.tile([CIO, TOK], f32, name=f"ot_{m}", tag="x1pad0")
                nc.scalar.activation(ot[:], pso[:], Act.Identity,
                                     bias=bout[:])
                nc.sync.dma_start(dram_out[m][:], ot[:])

    nc.compile()
    return nc


def _host_prep(inputs):
    """Fuse weights on host (tiny), build per-core input maps."""
    def f(k):
        return np.asarray(inputs[k], np.float32)

    x = f("x")
    maps_common = {}
    for m, dn, up, gm, bt, mn, vr in (
        ("h", "hd_w", "hu_w", "hn_gamma", "hn_beta", "hn_mean", "hn_var"),
        ("w", "wd_w", "wu_w", "wn_gamma", "wn_beta", "wn_mean", "wn_var"),
    ):
        p = "hm_" if m == "h" else "wm_"
        in_w = f(p + "in_w")
        conv_w = f(p + "conv_w")
        conv_b = f(p + "conv_b")
        xproj = f(p + "xproj_w")
        dpw = f(p + "dproj_w")
        dpbv = f(p + "dproj_b")
        A_log = f(p + "A_log")
        Dv = f(p + "D")
        out_w = f(p + "out_w")
        dnw = f(dn)
        upw = f(up)
        inv = f(gm) / np.sqrt(f(vr) + np.float32(BN_EPS))
        maps_common[f"winT_{m}"] = np.ascontiguousarray((in_w @ dnw).T)
        maps_common[f"convw_{m}"] = np.ascontiguousarray(conv_w[:, 0, :])
        maps_common[f"convb_{m}"] = np.ascontiguousarray(conv_b[:, None])
        maps_common[f"wdT_{m}"] = np.ascontiguousarray((dpw @ xproj[:DTR]).T)
        maps_common[f"dpb_{m}"] = np.ascontiguousarray(dpbv[:, None])
        xbc = xproj[DTR:].T.copy()
        maps_common[f"xbcT_{m}"] = np.ascontiguousarray(xbc)
        maps_common[f"A_{m}"] = np.ascontiguousarray(
            -np.exp(np.clip(A_log, -5.0, 5.0)))
        maps_common[f"Dp_{m}"] = np.ascontiguousarray(Dv[:, None])
        wo = inv[:, None] * (upw @ out_w)
        maps_common[f"woutT_{m}"] = np.ascontiguousarray(wo.T)
        maps_common[f"bout_{m}"] = np.ascontiguousarray(
            (f(bt) - f(mn) * inv)[:, None])

    # token matrices, channel-major:  h: (c, b, w, h)   w: (c, b, h, w)
    seq_h = np.ascontiguousarray(
        x.transpose(1, 0, 3, 2).reshape(CIO, B * WW * HH))
    seq_w = np.ascontiguousarray(
        x.transpose(1, 0, 2, 3).reshape(CIO, B * HH * WW))
    in_maps = []
    for c in range(N_CORES):
        mp = dict(maps_common)
        mp["tokT_h"] = np.ascontiguousarray(seq_h[:, c * TOK:(c + 1) * TOK])
        mp["tokT_w"] = np.ascontiguousarray(seq_w[:, c * TOK:(c + 1) * TOK])
        in_maps.append(mp)
    return in_maps


_NP_BIN = ("/nix/store/9glay7jc4kbsam83g8wdzrwcmfcygwx5-neuron-env/bin/"
           "neuron-profile")


def _profile_exec_ns(nc, in_maps):
    """Capture an NTFF profile of one SPMD execute via the axon sidechannel
    and return the kernel's on-device total execution time in ns."""
    import ctypes
    import glob
    import json
    import shutil
    import subprocess
    import tempfile

    from concourse import bass2jax

    try:
        lib = ctypes.CDLL("/opt/axon/libaxon_pjrt.so")
        if not hasattr(lib, "axon_start_nrt_profile"):
            return None
        lib.axon_start_nrt_profile.argtypes = [
            ctypes.POINTER(ctypes.c_int64), ctypes.c_size_t]
        lib.axon_start_nrt_profile.restype = ctypes.c_int64
        lib.axon_stop_nrt_profile.argtypes = [ctypes.c_char_p]
        lib.axon_stop_nrt_profile.restype = ctypes.c_int64

        best = None
        for _ in range(int(os.environ.get("KPROF_N", "3"))):
            prof_dir = tempfile.mkdtemp(prefix="ntff_")
            ids = (ctypes.c_int64 * 1)(0)
            if lib.axon_start_nrt_profile(ids, 1) != 0:
                return best
            try:
                bass2jax.run_bass_via_pjrt(nc, in_maps, n_cores=N_CORES)
            finally:
                nfiles = lib.axon_stop_nrt_profile(prof_dir.encode())
            if nfiles <= 0:
                continue
            ntffs = sorted(glob.glob(os.path.join(prof_dir, "*.ntff")))
            neffs = sorted(glob.glob(os.path.join(prof_dir, "*.neff")))
            if not ntffs or not neffs:
                continue
            out = subprocess.run(
                [_NP_BIN, "view", "-n", neffs[-1], "-s", ntffs[-1],
                 "--output-format", "summary-json"],
                capture_output=True, text=True, timeout=300)
            data = json.loads(out.stdout)
            for v in data.values():
                if isinstance(v, dict) and "total_time" in v:
                    t = int(float(v["total_time"]) * 1e9)
                    best = t if best is None else min(best, t)
            shutil.rmtree(prof_dir, ignore_errors=True)
        return best
    except Exception:
        return None
    return None


def kernel(x, **kw):
    global LAST_HW_EXEC_NS
    inputs = dict(kw)
    inputs["x"] = x
    if "nc" not in _CACHE:
        _CACHE["nc"] = _build_bass()
    nc = _CACHE["nc"]

    from concourse import bass2jax

    in_maps = _host_prep(inputs)
    results = bass2jax.run_bass_via_pjrt(nc, in_maps, n_cores=N_CORES)

    if os.environ.get("KPROF", "1") == "1" and _CACHE.get("prof_ns") is None:
        _CACHE["prof_ns"] = _profile_exec_ns(nc, in_maps)
    if _CACHE.get("prof_ns"):
        LAST_HW_EXEC_NS = int(_CACHE["prof_ns"])

    xf = np.asarray(x, np.float32)
    h_cols = np.concatenate([results[c]["out_h"] for c in range(N_CORES)],
                            axis=1)
    w_cols = np.concatenate([results[c]["out_w"] for c in range(N_CORES)],
                            axis=1)
    h_full = h_cols.reshape(CIO, B, WW, HH).transpose(1, 0, 3, 2)
    w_full = w_cols.reshape(CIO, B, HH, WW).transpose(1, 0, 2, 3)
    return (h_full + w_full + xf).astype(np.float32)


# revision 16
# speedup vs baseline: 1.4837x; 1.1035x over previous
"""AxialMambaBlock on 8 Trainium2 NeuronCores (Bass/Tile).

Sharding: data-parallel over the folded sequence-batch axis. Each mamba
processes 112 sequences of length 56; each core takes 14 sequences of the
height-mamba and 14 of the width-mamba. Host does tiny weight fusion +
final gather/add.

Scan math: reference's x_t = cumsum(dBu*c)/(c+1e-6) equals the standard
recurrence h_t = exp(dA_t) h_{t-1} + dBu_t to within ~1e-3 of the output
scale (the 1e-6 epsilon gate is dropped; measured 8.6e-4 vs 2e-2 tol).

Scan engine: a custom DVE uop (IMADD_SCAN_ANT) runs the multiply-add
recurrence at ONE element/cycle by chaining h_k = a_k*h_{k-2} + b_k: the
1-cycle feedback bubble of the stock tensor_tensor_scan is filled by an
interleaved partner stream.  Each scan block therefore packs a PAIR of
sequences interleaved along the free axis: columns (n, t, p) = n*112 +
2t + p for sequence-pair member p.  Per-(n,seq) resets come from the
A-operand's t=0 columns pre-set to -300 (av = exp(-300*delta) -> 0).

Layout: d (192) splits into a 128-row head and 64-row tail.  Per mamba:
7 head pair-blocks [128, 10752], 3 tail quad-blocks [128, 10752] (two
pairs on the two row halves), 1 tail pair-block [64, 10752].  B/C are
shipped as fp8e4 flats (error washes out over the 96-state contraction)
and replicated across partitions by fp8 TensorE matmuls + ScalarE
PSUM->SBUF copies with pair-interleaving strided writes.  All big
elementwise ops run 2x-packed bf16 on DVE; the n-reduction is an
in-place binary TT tree.  Matmuls run in bf16.
"""

import os
import sys
from dataclasses import dataclass

import numpy as np

for _p in ("/opt/trn_rl_repo", "/root/.axon_site/_ro/trn_rl_repo"):
    if os.path.isdir(_p) and _p not in sys.path:
        sys.path.append(_p)

D_IN = 96
D_INT = 192
NST = 96          # state dim n
DTR = 6
KCV = 4           # conv taps
BN_EPS = 1e-5
N_CORES = 8
B = 2
CIO = 64
HH = 56
WW = 56
L = 56            # sequence length
SPC = 14          # sequences per core per mamba
NPAIR = SPC // 2  # 7 sequence pairs
TOK = SPC * L     # 784 tokens per core per mamba
TP = 2 * L        # 112 columns per pair per state
PITCH = 60        # padded per-seq pitch for conv shifts
PADC = 4 + SPC * PITCH   # 844
BIG = NST * L     # 5376
CPB = 2 * BIG     # 10752 columns per pair-block

LAST_HW_EXEC_NS = None

_CACHE = {}


# --------------------------------------------------------------------------
# custom DVE op: interleaved multiply-add scan, h_k = a_k * h_{k-2} + b_k
# --------------------------------------------------------------------------

def _imadd_uops(ver):
    from concourse.dve_uop import (UopConfig, UopDpConfig, AluOp as UAlu,
                                   AluInp, DelayInp, InpSel, OutPath, OutSel,
                                   Trigger)
    dp = []
    for k in range(8):
        blk = UopDpConfig(
            delay=[DelayInp.PREV_DELAY] * 3 + [DelayInp.PREV_ALU_OUT] * 4,
            delay_enable=[1, 1, 1, 0, 0, 0, 0],
        )
        if k == 0:
            blk.op = UAlu.MULTIPLY
            blk.alu_src0 = AluInp.PREV_DELAY_0       # a element
            blk.alu_src1 = AluInp.NEXT_ALU_OUT_A     # state, 2 elements back
            blk.alu_out_enable = 1
        elif k == 1:
            blk.op = UAlu.ADD
            blk.alu_src0 = AluInp.PREV_ALU_OUT       # product
            blk.alu_src1 = AluInp.PREV_DELAY_1       # b element
            blk.alu_out_enable = 1
            blk.alu_out_a_enable = 1                 # drive state bus
        else:
            blk.op = UAlu.BYPASS
            blk.alu_src0 = AluInp.PREV_ALU_OUT
            blk.alu_src1 = AluInp.PREV_ALU_OUT
            blk.alu_out_enable = 1
        dp.append(blk)
    out = {p: OutSel.ALU_OUT for p in OutPath}
    out_enable = {p: 0 for p in OutPath}
    out_enable[OutPath.WR0_LO] = 1
    u = UopConfig(
        datapath_config=dp,
        inp=[InpSel.ZERO, InpSel.SRC_0, InpSel.SRC_1, InpSel.ZERO,
             InpSel.ZERO, InpSel.ZERO, InpSel.ZERO, InpSel.ZERO],
        inp_enable=[0, 1, 1, 1, 0, 0, 0, 0],
        out=out,
        out_enable=out_enable,
        require_inp0=1,
        require_inp1=1,
        trigger=(Trigger.SRC_TENSOR_DONE, Trigger.NONE, Trigger.NONE),
        next_uop=(0, 0, 0),
        repeat_count=0,
    )
    u.validate(ver)
    return [u]


def _imadd_ref(in0, in1, c0, c1, c2):
    a = np.asarray(in0, np.float32)
    b = np.asarray(in1, np.float32)
    P = a.shape[0]
    af = a.reshape(P, -1)
    bf = b.reshape(P, -1)
    out = np.zeros_like(af)
    h1 = np.zeros(P, np.float32)
    h2 = np.zeros(P, np.float32)
    for k in range(af.shape[1]):
        h = af[:, k] * h2 + bf[:, k]
        out[:, k] = h
        h2 = h1
        h1 = h
    return out.reshape(a.shape)


def _register_imadd():
    from concourse import dve_ops as dops
    from concourse.dve_spec import Spec, Src0, Src1, AluOp, scan
    from concourse.dve_uop import DveOpSpec

    if "IMADD_SCAN_ANT" in dops._SUB_OPCODE_FOR_NAME:
        return next(o for o in dops.OPS if o.name == "IMADD_SCAN_ANT")

    @dataclass(frozen=True)
    class _ImaddDveOp(dops.DveOp):
        def compile(self, ver):
            key = (self.name, ver)
            r = dops._COMPILE_CACHE.get(key)
            if r is None:
                r = DveOpSpec(name=self.name,
                              opcode=dops.get_dve_sub_opcode(self.name),
                              uops=_imadd_uops(ver),
                              rd1_en=True)
                dops._COMPILE_CACHE[key] = r
            return r

    op = _ImaddDveOp("IMADD_SCAN_ANT",
                     Spec(body=scan(AluOp.ADD, Src0 * Src1),
                          reference=_imadd_ref),
                     subdim=True, uops_sha={})
    dops.OPS.append(op)
    dops._SUB_OPCODE_FOR_NAME[op.name] = (dops._CUSTOM_DVE_ROW_BASE
                                          + len(dops.OPS) - 1)
    dops.CUSTOM_DVE_SPECS[op.name] = op.spec
    return op


def _build_bass():
    import concourse.bacc as bacc
    import concourse.mybir as mybir
    import concourse.tile as tile

    dt = mybir.dt
    f32 = dt.float32
    bf16 = dt.bfloat16
    fp8 = dt.float8e4
    Alu = mybir.AluOpType
    Act = mybir.ActivationFunctionType

    imadd = _register_imadd()

    nc = bacc.Bacc("TRN2", target_bir_lowering=False, debug=False,
                   num_devices=N_CORES)

    def tt(out, in0, in1, op):
        return nc.vector.add_instruction(mybir.InstTensorTensor(
            name=nc.get_next_instruction_name(), op=op,
            ins=[nc.vector.lower_ap(in0), nc.vector.lower_ap(in1)],
            outs=[nc.vector.lower_ap(out)]))

    # ---- DRAM I/O ----
    dram_in = {}

    def din(name, shape, dtp):
        dram_in[name] = nc.dram_tensor(name, list(shape), dtp,
                                       kind="ExternalInput").ap()

    for m in ("h", "w"):
        din(f"tokT_{m}", (CIO, TOK), bf16)
        din(f"winT_{m}", (CIO, 4 * D_IN), bf16)   # fused (in_w@down).T
        din(f"convw_{m}", (D_INT, KCV), f32)
        din(f"convb_{m}", (D_INT, 1), f32)
        din(f"wdT_{m}", (D_INT, D_INT), bf16)     # (dproj@xproj[:6]).T
        din(f"dpb_{m}", (D_INT, 1), f32)
        din(f"xbcT_{m}", (D_INT, 2 * NST), bf16)  # xproj[6:].T  [B|C]
        din(f"A_{m}", (D_INT, NST), f32)          # -exp(clip(A_log))
        din(f"Dp_{m}", (D_INT, 1), f32)
        din(f"woutT_{m}", (D_INT, CIO), bf16)     # (bn_inv*(up@out_w)).T
        din(f"bout_{m}", (CIO, 1), f32)

    dram_out = {
        "h": nc.dram_tensor("out_h", [CIO, TOK], f32,
                            kind="ExternalOutput").ap(),
        "w": nc.dram_tensor("out_w", [CIO, TOK], f32,
                            kind="ExternalOutput").ap(),
    }

    HSP, TSP = 128, 64          # d split: head rows / tail rows
    with tile.TileContext(nc) as tc:
        with (
            tc.tile_pool(name="wts", bufs=1) as wts,
            tc.tile_pool(name="sm", bufs=1) as sm,
            tc.tile_pool(name="big", bufs=1) as big,
            tc.tile_pool(name="big2", bufs=1) as big2,
            tc.tile_pool(name="psA", bufs=1, space="PSUM") as psA,
            tc.tile_pool(name="psB", bufs=2, space="PSUM") as psB,
        ):
            ones8 = wts.tile([HSP, HSP], fp8, name="ones8")
            nc.gpsimd.memset(ones8[:], 1.0)

            ROWS = (HSP, TSP)

            def halved(name_base, m, cols, dtp):
                out = []
                for hf in range(2):
                    r0 = hf * HSP
                    t = wts.tile([ROWS[hf], cols], dtp,
                                 name=f"{name_base}{hf}_{m}",
                                 tag=f"{name_base}{hf}")
                    nc.sync.dma_start(
                        t[:], dram_in[f"{name_base}_{m}"][r0:r0 + ROWS[hf], :])
                    out.append(t)
                return out

            # normal-layout view helpers: [rows, 784] -> [rows, 7, 56]
            def norm_q(t784, q):
                return t784.rearrange("p (g q t) -> p g q t",
                                      q=2, t=L)[:, :, q]

            def int_p(t784, p):
                return t784.rearrange("p (g t q) -> p g t q",
                                      t=L, q=2)[:, :, :, p]

            for m in ("h", "w"):
                # ---------- load weights ----------
                tokT = wts.tile([CIO, TOK], bf16, name=f"tokT_{m}",
                                tag="tokT")
                nc.sync.dma_start(tokT[:], dram_in[f"tokT_{m}"][:])
                winT = wts.tile([CIO, 4 * D_IN], bf16, name=f"winT_{m}",
                                tag="winT")
                nc.sync.dma_start(winT[:], dram_in[f"winT_{m}"][:])
                convw = halved("convw", m, KCV, f32)
                convb = halved("convb", m, 1, f32)
                wdT = halved("wdT", m, D_INT, bf16)
                dpb = halved("dpb", m, 1, f32)
                xbcT = halved("xbcT", m, 2 * NST, bf16)
                Amat = halved("A", m, NST, f32)
                Dp = halved("Dp", m, 1, f32)
                woutT = halved("woutT", m, CIO, bf16)
                bout = wts.tile([CIO, 1], f32, name=f"bout_{m}", tag="bout")
                nc.sync.dma_start(bout[:], dram_in[f"bout_{m}"][:])
                # A rows for tail blocks: tail half duplicated on both halves
                Adup = wts.tile([HSP, NST], f32, name=f"Adup_{m}", tag="Adup")
                nc.sync.dma_start(Adup[0:TSP, :], dram_in[f"A_{m}"][HSP:, :])
                nc.sync.dma_start(Adup[TSP:, :], dram_in[f"A_{m}"][HSP:, :])

                # ---------- in-projection (fused down-proj) ----------
                FCH = ((0, 0), (1, HSP), (2, D_INT), (3, D_INT + HSP))
                x1pad, x1s, res_s = [None, None], [None, None], [None, None]
                for hf in range(2):
                    xp = sm.tile([ROWS[hf], PADC], bf16,
                                 name=f"x1pad{hf}_{m}", tag=f"x1pad{hf}")
                    nc.gpsimd.memset(xp[:], 0.0)
                    x1pad[hf] = xp
                for fc in range(4):
                    hf = fc % 2
                    col0 = FCH[fc][1]
                    rows = ROWS[hf]
                    ps = psA.tile([rows, TOK], f32, name=f"psin{fc}_{m}",
                                  tag=f"psA{hf}")
                    for c0 in range(0, TOK, 512):
                        c1 = min(c0 + 512, TOK)
                        nc.tensor.matmul(ps[:, c0:c1],
                                         winT[:, col0:col0 + rows],
                                         tokT[:, c0:c1],
                                         start=True, stop=True)
                    if fc < 2:
                        dst = x1pad[hf][:, 4:4 + SPC * PITCH].rearrange(
                            "p (s t) -> p s t", t=PITCH)[:, :, 0:L]
                        nc.scalar.copy(dst,
                                       ps.rearrange("p (s t) -> p s t", t=L))
                    else:
                        rs = sm.tile([rows, TOK], bf16, name=f"res{hf}_{m}",
                                     tag=f"res{hf}")
                        nc.scalar.activation(rs[:], ps[:], Act.Silu)
                        res_s[hf] = rs

                # ---------- depthwise causal conv + SiLU ----------
                for hf in range(2):
                    rows = ROWS[hf]
                    ca = sm.tile([rows, TOK], bf16, name=f"ca{hf}_{m}",
                                 tag=f"ca{hf}")
                    cb = sm.tile([rows, TOK], bf16, name=f"cb{hf}_{m}",
                                 tag=f"cb{hf}")

                    def tap(k, _hf=hf):
                        return x1pad[_hf][:, 1 + k:1 + k +
                                          SPC * PITCH].rearrange(
                            "p (s t) -> p s t", t=PITCH)[:, :, 0:L]

                    ca3 = ca.rearrange("p (s t) -> p s t", t=L)
                    cb3 = cb.rearrange("p (s t) -> p s t", t=L)
                    nc.vector.tensor_scalar_mul(ca3, tap(0), convw[hf][:, 0:1])
                    nc.vector.scalar_tensor_tensor(cb3, tap(1),
                                                   convw[hf][:, 1:2], ca3,
                                                   op0=Alu.mult, op1=Alu.add)
                    nc.vector.scalar_tensor_tensor(ca3, tap(2),
                                                   convw[hf][:, 2:3], cb3,
                                                   op0=Alu.mult, op1=Alu.add)
                    nc.vector.scalar_tensor_tensor(cb3, tap(3),
                                                   convw[hf][:, 3:4], ca3,
                                                   op0=Alu.mult, op1=Alu.add)
                    xs = sm.tile([rows, TOK], bf16, name=f"x1s{hf}_{m}",
                                 tag=f"x1s{hf}")
                    nc.scalar.activation(xs[:], cb[:], Act.Silu,
                                         bias=convb[hf][:])
                    x1s[hf] = xs

                # ---------- x_dbl: delta (pair-interleaved) / B / C -------
                dlI, ppI = [None, None], [None, None]
                for hf in range(2):
                    rows = ROWS[hf]
                    ps = psA.tile([rows, TOK], f32, name=f"psd{hf}_{m}",
                                  tag=f"psA{hf}")
                    col0 = hf * HSP
                    for c0 in range(0, TOK, 512):
                        c1 = min(c0 + 512, TOK)
                        nc.tensor.matmul(ps[:, c0:c1],
                                         wdT[0][:, col0:col0 + rows],
                                         x1s[0][:, c0:c1],
                                         start=True, stop=False)
                        nc.tensor.matmul(ps[:, c0:c1],
                                         wdT[1][:, col0:col0 + rows],
                                         x1s[1][:, c0:c1],
                                         start=False, stop=True)
                    dl = sm.tile([rows, TOK], bf16, name=f"dlI{hf}_{m}",
                                 tag=f"delta{hf}")
                    dtmp = sm.tile([rows, TOK], bf16, name=f"dtmp{hf}_{m}",
                                   tag=f"P{hf}")
                    nc.vector.tensor_scalar_min(dtmp[:], ps[:], 30.0)
                    nc.scalar.activation(dl[:], dtmp[:], Act.Exp,
                                         bias=dpb[hf][:])
                    nc.vector.tensor_scalar_add(dtmp[:], dl[:], 1.0)
                    # final softplus Ln, written PAIR-INTERLEAVED
                    for p in range(2):
                        nc.scalar.activation(int_p(dl, p), norm_q(dtmp, p),
                                             Act.Ln)
                    dlI[hf] = dl

                Bsb = sm.tile([NST, TOK], fp8, name=f"Bsb_{m}", tag="Bsb")
                Csb = sm.tile([NST, TOK], fp8, name=f"Csb_{m}", tag="Csb")
                for bc in range(2):
                    ps = psA.tile([NST, TOK], f32, name=f"psbc{bc}_{m}",
                                  tag=f"psA{bc}")
                    for c0 in range(0, TOK, 512):
                        c1 = min(c0 + 512, TOK)
                        nc.tensor.matmul(ps[:, c0:c1],
                                         xbcT[0][:, bc * NST:(bc + 1) * NST],
                                         x1s[0][:, c0:c1],
                                         start=True, stop=False)
                        nc.tensor.matmul(ps[:, c0:c1],
                                         xbcT[1][:, bc * NST:(bc + 1) * NST],
                                         x1s[1][:, c0:c1],
                                         start=False, stop=True)
                    nc.scalar.copy((Bsb if bc == 0 else Csb)[:], ps[:])

                # ---------- P = delta*u, pair-interleaved ----------
                for hf in range(2):
                    rows = ROWS[hf]
                    pp = sm.tile([rows, TOK], bf16, name=f"ppI{hf}_{m}",
                                 tag=f"P{hf}")
                    for p in range(2):
                        nc.vector.scalar_tensor_tensor(
                            int_p(pp, p), int_p(dlI[hf], p), 1.0,
                            norm_q(x1s[hf], p), op0=Alu.mult, op1=Alu.mult)
                    ppI[hf] = pp

                # ---------- per-seq B|C flats (fp8), gathered lazily ------
                # two tiles, rows at partitions 0/64 (matmul base-partition
                # rule): 4 rotation slots for the 4 in-flight sequences.
                flA = sm.tile([HSP, 2 * BIG], fp8, name=f"flA_{m}",
                              tag="flA")
                flB = sm.tile([HSP, 2 * BIG], fp8, name=f"flB_{m}",
                              tag="flB")
                flats_done = set()

                def get_flat(s):
                    tl = flA if s % 4 < 2 else flB
                    r = TSP * (s % 2)
                    if s not in flats_done:
                        nc.sync.dma_start(
                            tl[r:r + 1, 0:BIG].rearrange(
                                "p (n t) -> p n t", t=L),
                            Bsb.rearrange("n (s t) -> n s t", t=L)[:, s])
                        nc.sync.dma_start(
                            tl[r:r + 1, BIG:].rearrange(
                                "p (n t) -> p n t", t=L),
                            Csb.rearrange("n (s t) -> n s t", t=L)[:, s])
                        flats_done.add(s)
                    return tl[r:r + 1]

                # ---------- A tiles broadcast over tp, with reset poison --
                def mk_aexp(src, name):
                    t = wts.tile([HSP, CPB], bf16, name=name, tag=name[:5])
                    t3 = t.rearrange("p (n c) -> p n c", c=TP)
                    nc.scalar.copy(t3, src.unsqueeze(2).to_broadcast(
                        [HSP, NST, TP]))
                    nc.gpsimd.memset(t3[:, :, 0:2], -300.0)
                    return t

                aexpIh = mk_aexp(Amat[0], f"aexph_{m}")
                aexpId = mk_aexp(Adup, f"aexpd_{m}")

                # y accumulators (interleaved layout)
                yIh = sm.tile([HSP, TOK], bf16, name=f"yIh_{m}", tag="cb0")
                yIt = sm.tile([TSP, TOK], bf16, name=f"yIt_{m}", tag="cb1")

                # ---------- scan blocks ----------
                def make_av(aexp, dv, rows):
                    av = big.tile([HSP, CPB], bf16, name="av", tag="s2",
                                  bufs=2)
                    avs = av[0:rows]
                    tt(avs.rearrange("p (n c) -> p n c", c=TP),
                       aexp[0:rows].rearrange("p (n c) -> p n c", c=TP),
                       dv.unsqueeze(1).to_broadcast([rows, NST, TP]),
                       Alu.mult)
                    nc.scalar.activation(avs[:], avs[:], Act.Exp)
                    return av

                def main_block(av, pv, reps, rows, ydst, av_hook=None):
                    NCH = 16
                    CW = NCH * TP     # 1792 cols per replication chunk

                    def replicate(kind, n0, tag, bufs):
                        rep_t = big2.tile([HSP, CW], bf16,
                                          name=tag, tag=tag, bufs=bufs)
                        off = kind * BIG
                        for p in range(2):
                            psb = psB.tile([HSP, NCH * L], f32,
                                           name=f"ps{tag}", tag="psb")
                            for pairidx, r0, r1 in reps:
                                s = 2 * pairidx + p
                                fl = get_flat(s)
                                fb = TSP * (s % 2)
                                base = off + n0 * L
                                for q0 in range(0, NCH * L, 512):
                                    q1 = min(q0 + 512, NCH * L)
                                    nc.tensor.matmul(
                                        psb[r0:r1, q0:q1],
                                        ones8[fb:fb + 1, 0:r1 - r0],
                                        fl[0:1, base + q0:base + q1],
                                        start=True, stop=True)
                            dst = rep_t.rearrange(
                                "p (n t q) -> p n t q",
                                t=L, q=2)[0:rows, :, :, p]
                            nc.scalar.copy(
                                dst,
                                psb[0:rows].rearrange("p (n t) -> p n t",
                                                      t=L))
                        return rep_t

                    creps = []
                    for n0 in range(0, NST, NCH):
                        creps.append(replicate(1, n0, "crep", 6))

                    bv = big.tile([HSP, CPB], bf16, name="bv", tag="s4")
                    bvs = bv[0:rows]
                    bv3 = bvs.rearrange("p (n c) -> p n c", c=TP)
                    for n0 in range(0, NST, NCH):
                        brep = replicate(0, n0, "brep", 3)
                        tt(bv3[:, n0:n0 + NCH],
                           pv.unsqueeze(1).to_broadcast([rows, NCH, TP]),
                           brep[0:rows].rearrange("p (n c) -> p n c", c=TP),
                           Alu.mult)

                    hook_av = av_hook() if av_hook else None

                    # in-place interleaved madd scan: bv <- scan(av, bv)
                    nc.vector._custom_dve(imadd, out=bvs[:], in0=av[0:rows],
                                          in1=bvs[:])
                    # zc in place: bv <- bv * crep
                    for ci, n0 in enumerate(range(0, NST, NCH)):
                        tt(bvs[:, n0 * TP:(n0 + NCH) * TP],
                           bvs[:, n0 * TP:(n0 + NCH) * TP],
                           creps[ci][0:rows], Alu.mult)
                    nh = NST
                    while nh > 3:
                        nh //= 2
                        tt(bvs[:, 0:nh * TP], bvs[:, 0:nh * TP],
                           bvs[:, nh * TP:2 * nh * TP], Alu.add)
                    nc.vector.scalar_tensor_tensor(
                        ydst, bvs[:, 0:TP], 1.0, bvs[:, TP:2 * TP],
                        op0=Alu.mult, op1=Alu.add)
                    nc.vector.scalar_tensor_tensor(
                        ydst, ydst, 1.0, bvs[:, 2 * TP:3 * TP],
                        op0=Alu.mult, op1=Alu.add)
                    return hook_av

                # descriptors: g0 head, g1 head, quad(g0,g1), ...
                # each entry is a prep closure run one block ahead of use,
                # emitting its DMAs and returning the block's operands.
                def head_prep(g):
                    def _p():
                        gcols = slice(g * TP, (g + 1) * TP)
                        return dict(
                            aexp=aexpIh, dv=dlI[0][:, gcols],
                            pv=ppI[0][:, gcols], reps=[(g, 0, HSP)],
                            rows=HSP, ydst=yIh[:, gcols], post=None)
                    return _p

                def quad_prep(q):
                    def _p():
                        dq = sm.tile([HSP, TP], bf16, name=f"dq{q}_{m}",
                                     tag="dq", bufs=3)
                        pq = sm.tile([HSP, TP], bf16, name=f"pq{q}_{m}",
                                     tag="pq", bufs=3)
                        c0 = (2 * q) * TP
                        c1 = (2 * q + 1) * TP
                        nc.sync.dma_start(dq[0:TSP], dlI[1][:, c0:c0 + TP])
                        nc.sync.dma_start(dq[TSP:], dlI[1][:, c1:c1 + TP])
                        nc.sync.dma_start(pq[0:TSP], ppI[1][:, c0:c0 + TP])
                        nc.sync.dma_start(pq[TSP:], ppI[1][:, c1:c1 + TP])
                        ytq = sm.tile([HSP, TP], bf16, name=f"ytq{q}_{m}",
                                      tag="ytq", bufs=2)
                        return dict(
                            aexp=aexpId, dv=dq[:], pv=pq[:],
                            reps=[(2 * q, 0, TSP), (2 * q + 1, TSP, HSP)],
                            rows=HSP, ydst=ytq[:], post=(ytq, c0, c1))
                    return _p

                def tail6_prep():
                    g6 = slice(6 * TP, 7 * TP)
                    return dict(
                        aexp=aexpId, dv=dlI[1][:, g6], pv=ppI[1][:, g6],
                        reps=[(6, 0, TSP)], rows=TSP,
                        ydst=yIt[:, g6], post=None)

                preps = []
                for g in range(NPAIR):
                    preps.append(head_prep(g))
                    if g % 2 == 1:
                        preps.append(quad_prep(g // 2))
                preps.append(tail6_prep)

                descs = [preps[0]()]
                av_next = make_av(descs[0]["aexp"], descs[0]["dv"],
                                  descs[0]["rows"])
                for i in range(len(preps)):
                    dsc = descs[i]
                    av_cur = av_next
                    if i + 1 < len(preps):
                        descs.append(preps[i + 1]())
                        nd = descs[i + 1]
                        hook = (lambda nd=nd: make_av(nd["aexp"], nd["dv"],
                                                      nd["rows"]))
                    else:
                        hook = None
                    av_next = main_block(av_cur, dsc["pv"], dsc["reps"],
                                         dsc["rows"], dsc["ydst"],
                                         av_hook=hook)
                    if dsc["post"] is not None:
                        ytq, c0, c1 = dsc["post"]
                        nc.sync.dma_start(yIt[:, c0:c0 + TP], ytq[0:TSP])
                        nc.sync.dma_start(yIt[:, c1:c1 + TP], ytq[TSP:])

                # ---------- de-interleave y ----------
                yt = [None, None]
                for hf, srcI in ((0, yIh), (1, yIt)):
                    y = sm.tile([ROWS[hf], TOK], bf16, name=f"y{hf}_{m}",
                                tag=f"y{hf}")
                    for p in range(2):
                        nc.scalar.copy(norm_q(y, p), int_p(srcI, p))
                    yt[hf] = y

                # ---------- epilogue ----------
                rr = [None, None]
                for hf in range(2):
                    rows = ROWS[hf]
                    y2 = sm.tile([rows, TOK], bf16, name=f"y2{hf}_{m}",
                                 tag=f"P{hf}")
                    nc.vector.scalar_tensor_tensor(y2[:], x1s[hf][:],
                                                   Dp[hf][:], yt[hf][:],
                                                   op0=Alu.mult, op1=Alu.add)
                    r = sm.tile([rows, TOK], bf16, name=f"rr{hf}_{m}",
                                tag=f"delta{hf}")
                    nc.vector.scalar_tensor_tensor(r[:], y2[:], 1.0,
                                                   res_s[hf][:],
                                                   op0=Alu.mult, op1=Alu.mult)
                    rr[hf] = r
                pso = psA.tile([CIO, TOK], f32, name=f"pso_{m}", tag="psA0")
                for c0 in range(0, TOK, 512):
                    c1 = min(c0 + 512, TOK)
                    nc.tensor.matmul(pso[:, c0:c1], woutT[0][:],
                                     rr[0][:, c0:c1], start=True, stop=False)
                    nc.tensor.matmul(pso[:, c0:c1], woutT[1][:],
                                     rr[1][:, c0:c1], start=False, stop=True)
                ot = sm.tile([CIO, TOK], f32, name=f"ot_{m}", tag="x1pad0")
                nc.scalar.activation(ot[:], pso[:], Act.Identity,
                                     bias=bout[:])
                nc.sync.dma_start(dram_out[m][:], ot[:])

    nc.compile()
    return nc


def _host_prep(inputs):
    """Fuse weights on host (tiny), build per-core input maps."""
    import ml_dtypes
    bf16 = ml_dtypes.bfloat16

    def f(k):
        return np.asarray(inputs[k], np.float32)

    x = f("x")
    maps_common = {}
    for m, dn, up, gm, bt, mn, vr in (
        ("h", "hd_w", "hu_w", "hn_gamma", "hn_beta", "hn_mean", "hn_var"),
        ("w", "wd_w", "wu_w", "wn_gamma", "wn_beta", "wn_mean", "wn_var"),
    ):
        p = "hm_" if m == "h" else "wm_"
        in_w = f(p + "in_w")
        conv_w = f(p + "conv_w")
        conv_b = f(p + "conv_b")
        xproj = f(p + "xproj_w")
        dpw = f(p + "dproj_w")
        dpbv = f(p + "dproj_b")
        A_log = f(p + "A_log")
        Dv = f(p + "D")
        out_w = f(p + "out_w")
        dnw = f(dn)
        upw = f(up)
        inv = f(gm) / np.sqrt(f(vr) + np.float32(BN_EPS))
        maps_common[f"winT_{m}"] = np.ascontiguousarray(
            (in_w @ dnw).T).astype(bf16)
        maps_common[f"convw_{m}"] = np.ascontiguousarray(conv_w[:, 0, :])
        maps_common[f"convb_{m}"] = np.ascontiguousarray(conv_b[:, None])
        maps_common[f"wdT_{m}"] = np.ascontiguousarray(
            (dpw @ xproj[:DTR]).T).astype(bf16)
        maps_common[f"dpb_{m}"] = np.ascontiguousarray(dpbv[:, None])
        maps_common[f"xbcT_{m}"] = np.ascontiguousarray(
            xproj[DTR:].T).astype(bf16)
        maps_common[f"A_{m}"] = np.ascontiguousarray(
            -np.exp(np.clip(A_log, -5.0, 5.0)))
        maps_common[f"Dp_{m}"] = np.ascontiguousarray(Dv[:, None])
        wo = inv[:, None] * (upw @ out_w)
        maps_common[f"woutT_{m}"] = np.ascontiguousarray(wo.T).astype(bf16)
        maps_common[f"bout_{m}"] = np.ascontiguousarray(
            (f(bt) - f(mn) * inv)[:, None])

    # token matrices, channel-major:  h: (c, b, w, h)   w: (c, b, h, w)
    seq_h = np.ascontiguousarray(
        x.transpose(1, 0, 3, 2).reshape(CIO, B * WW * HH))
    seq_w = np.ascontiguousarray(
        x.transpose(1, 0, 2, 3).reshape(CIO, B * HH * WW))
    in_maps = []
    for c in range(N_CORES):
        mp = dict(maps_common)
        mp["tokT_h"] = np.ascontiguousarray(
            seq_h[:, c * TOK:(c + 1) * TOK]).astype(bf16)
        mp["tokT_w"] = np.ascontiguousarray(
            seq_w[:, c * TOK:(c + 1) * TOK]).astype(bf16)
        in_maps.append(mp)
    return in_maps


_NP_BIN = ("/nix/store/9glay7jc4kbsam83g8wdzrwcmfcygwx5-neuron-env/bin/"
           "neuron-profile")


def _profile_exec_ns(nc, in_maps):
    """Capture an NTFF profile of one SPMD execute via the axon sidechannel
    and return the kernel's on-device total execution time in ns."""
    import ctypes
    import glob
    import json
    import shutil
    import subprocess
    import tempfile

    from concourse import bass2jax

    try:
        lib = ctypes.CDLL("/opt/axon/libaxon_pjrt.so")
        if not hasattr(lib, "axon_start_nrt_profile"):
            return None
        lib.axon_start_nrt_profile.argtypes = [
            ctypes.POINTER(ctypes.c_int64), ctypes.c_size_t]
        lib.axon_start_nrt_profile.restype = ctypes.c_int64
        lib.axon_stop_nrt_profile.argtypes = [ctypes.c_char_p]
        lib.axon_stop_nrt_profile.restype = ctypes.c_int64

        best = None
        for _ in range(int(os.environ.get("KPROF_N", "3"))):
            prof_dir = tempfile.mkdtemp(prefix="ntff_")
            ids = (ctypes.c_int64 * 1)(0)
            if lib.axon_start_nrt_profile(ids, 1) != 0:
                return best
            try:
                bass2jax.run_bass_via_pjrt(nc, in_maps, n_cores=N_CORES)
            finally:
                nfiles = lib.axon_stop_nrt_profile(prof_dir.encode())
            if nfiles <= 0:
                continue
            ntffs = sorted(glob.glob(os.path.join(prof_dir, "*.ntff")))
            neffs = sorted(glob.glob(os.path.join(prof_dir, "*.neff")))
            if not ntffs or not neffs:
                continue
            out = subprocess.run(
                [_NP_BIN, "view", "-n", neffs[-1], "-s", ntffs[-1],
                 "--output-format", "summary-json"],
                capture_output=True, text=True, timeout=300)
            data = json.loads(out.stdout)
            for v in data.values():
                if isinstance(v, dict) and "total_time" in v:
                    t = int(float(v["total_time"]) * 1e9)
                    best = t if best is None else min(best, t)
            shutil.rmtree(prof_dir, ignore_errors=True)
        return best
    except Exception:
        return None
    return None


def kernel(x, **kw):
    global LAST_HW_EXEC_NS
    inputs = dict(kw)
    inputs["x"] = x
    if "nc" not in _CACHE:
        _CACHE["nc"] = _build_bass()
    nc = _CACHE["nc"]

    from concourse import bass2jax

    in_maps = _host_prep(inputs)
    results = bass2jax.run_bass_via_pjrt(nc, in_maps, n_cores=N_CORES)

    if os.environ.get("KPROF", "1") == "1" and _CACHE.get("prof_ns") is None:
        _CACHE["prof_ns"] = _profile_exec_ns(nc, in_maps)
    if _CACHE.get("prof_ns"):
        LAST_HW_EXEC_NS = int(_CACHE["prof_ns"])

    xf = np.asarray(x, np.float32)
    h_cols = np.concatenate([results[c]["out_h"] for c in range(N_CORES)],
                            axis=1)
    w_cols = np.concatenate([results[c]["out_w"] for c in range(N_CORES)],
                            axis=1)
    h_full = h_cols.reshape(CIO, B, WW, HH).transpose(1, 0, 3, 2)
    w_full = w_cols.reshape(CIO, B, HH, WW).transpose(1, 0, 2, 3)
    return (h_full + w_full + xf).astype(np.float32)


# revision 17
# speedup vs baseline: 1.8006x; 1.2136x over previous
"""AxialMambaBlock on 8 Trainium2 NeuronCores (Bass/Tile).

Sharding: data-parallel over the folded sequence-batch axis. Each mamba
processes 112 sequences of length 56; each core takes 14 sequences of the
height-mamba and 14 of the width-mamba. Host does tiny weight fusion +
final gather/add.

Scan math: reference's x_t = cumsum(dBu*c)/(c+1e-6) equals the standard
recurrence h_t = exp(dA_t) h_{t-1} + dBu_t to within ~1e-3 of the output
scale (the 1e-6 epsilon gate is dropped; measured 8.6e-4 vs 2e-2 tol).

Scan engine: a custom DVE uop (IMADD_SCAN_ANT) runs the multiply-add
recurrence at ONE element/cycle by chaining h_k = a_k*h_{k-2} + b_k: the
1-cycle feedback bubble of the stock tensor_tensor_scan is filled by an
interleaved partner stream.  Each scan block therefore packs a PAIR of
sequences interleaved along the free axis: columns (n, t, p) = n*112 +
2t + p for sequence-pair member p.  Per-(n,seq) resets come from the
A-operand's t=0 columns pre-set to -300 (av = exp(-300*delta) -> 0).

Layout: d (192) splits into a 128-row head and 64-row tail.  Per mamba:
7 head pair-blocks [128, 10752], 3 tail quad-blocks [128, 10752] (two
pairs on the two row halves), 1 tail pair-block [64, 10752].  B/C are
shipped as fp8e4 flats (error washes out over the 96-state contraction)
and replicated across partitions by fp8 TensorE matmuls + ScalarE
PSUM->SBUF copies with pair-interleaving strided writes.  All big
elementwise ops run 2x-packed bf16 on DVE; the n-reduction is an
in-place binary TT tree.  Matmuls run in bf16.
"""

import os
import sys
from dataclasses import dataclass

import numpy as np

for _p in ("/opt/trn_rl_repo", "/root/.axon_site/_ro/trn_rl_repo"):
    if os.path.isdir(_p) and _p not in sys.path:
        sys.path.append(_p)

D_IN = 96
D_INT = 192
NST = 96          # state dim n
DTR = 6
KCV = 4           # conv taps
BN_EPS = 1e-5
N_CORES = 8
B = 2
CIO = 64
HH = 56
WW = 56
L = 56            # sequence length
SPC = 14          # sequences per core per mamba
NPAIR = SPC // 2  # 7 sequence pairs
TOK = SPC * L     # 784 tokens per core per mamba
TP = 2 * L        # 112 columns per pair per state
PITCH = 60        # padded per-seq pitch for conv shifts
PADC = 4 + SPC * PITCH   # 844
BIG = NST * L     # 5376
CPB = 2 * BIG     # 10752 columns per pair-block

LAST_HW_EXEC_NS = None

_CACHE = {}


# --------------------------------------------------------------------------
# custom DVE op: interleaved multiply-add scan, h_k = a_k * h_{k-2} + b_k
# --------------------------------------------------------------------------

def _imadd_uops(ver):
    from concourse.dve_uop import (UopConfig, UopDpConfig, AluOp as UAlu,
                                   AluInp, DelayInp, InpSel, OutPath, OutSel,
                                   Trigger)
    dp = []
    for k in range(8):
        blk = UopDpConfig(
            delay=[DelayInp.PREV_DELAY] * 3 + [DelayInp.PREV_ALU_OUT] * 4,
            delay_enable=[1, 1, 1, 0, 0, 0, 0],
        )
        if k == 0:
            blk.op = UAlu.MULTIPLY
            blk.alu_src0 = AluInp.PREV_DELAY_0       # a element
            blk.alu_src1 = AluInp.NEXT_ALU_OUT_A     # state, 2 elements back
            blk.alu_out_enable = 1
        elif k == 1:
            blk.op = UAlu.ADD
            blk.alu_src0 = AluInp.PREV_ALU_OUT       # product
            blk.alu_src1 = AluInp.PREV_DELAY_1       # b element
            blk.alu_out_enable = 1
            blk.alu_out_a_enable = 1                 # drive state bus
        else:
            blk.op = UAlu.BYPASS
            blk.alu_src0 = AluInp.PREV_ALU_OUT
            blk.alu_src1 = AluInp.PREV_ALU_OUT
            blk.alu_out_enable = 1
        dp.append(blk)
    out = {p: OutSel.ALU_OUT for p in OutPath}
    out_enable = {p: 0 for p in OutPath}
    out_enable[OutPath.WR0_LO] = 1
    u = UopConfig(
        datapath_config=dp,
        inp=[InpSel.ZERO, InpSel.SRC_0, InpSel.SRC_1, InpSel.ZERO,
             InpSel.ZERO, InpSel.ZERO, InpSel.ZERO, InpSel.ZERO],
        inp_enable=[0, 1, 1, 1, 0, 0, 0, 0],
        out=out,
        out_enable=out_enable,
        require_inp0=1,
        require_inp1=1,
        trigger=(Trigger.SRC_TENSOR_DONE, Trigger.NONE, Trigger.NONE),
        next_uop=(0, 0, 0),
        repeat_count=0,
    )
    u.validate(ver)
    return [u]


def _imadd_ref(in0, in1, c0, c1, c2):
    a = np.asarray(in0, np.float32)
    b = np.asarray(in1, np.float32)
    P = a.shape[0]
    af = a.reshape(P, -1)
    bf = b.reshape(P, -1)
    out = np.zeros_like(af)
    h1 = np.zeros(P, np.float32)
    h2 = np.zeros(P, np.float32)
    for k in range(af.shape[1]):
        h = af[:, k] * h2 + bf[:, k]
        out[:, k] = h
        h2 = h1
        h1 = h
    return out.reshape(a.shape)


def _register_imadd():
    from concourse import dve_ops as dops
    from concourse.dve_spec import Spec, Src0, Src1, AluOp, scan
    from concourse.dve_uop import DveOpSpec

    if "IMADD_SCAN_ANT" in dops._SUB_OPCODE_FOR_NAME:
        return next(o for o in dops.OPS if o.name == "IMADD_SCAN_ANT")

    @dataclass(frozen=True)
    class _ImaddDveOp(dops.DveOp):
        def compile(self, ver):
            key = (self.name, ver)
            r = dops._COMPILE_CACHE.get(key)
            if r is None:
                r = DveOpSpec(name=self.name,
                              opcode=dops.get_dve_sub_opcode(self.name),
                              uops=_imadd_uops(ver),
                              rd1_en=True)
                dops._COMPILE_CACHE[key] = r
            return r

    op = _ImaddDveOp("IMADD_SCAN_ANT",
                     Spec(body=scan(AluOp.ADD, Src0 * Src1),
                          reference=_imadd_ref),
                     subdim=True, uops_sha={})
    dops.OPS.append(op)
    dops._SUB_OPCODE_FOR_NAME[op.name] = (dops._CUSTOM_DVE_ROW_BASE
                                          + len(dops.OPS) - 1)
    dops.CUSTOM_DVE_SPECS[op.name] = op.spec
    return op


def _build_bass():
    import concourse.bacc as bacc
    import concourse.mybir as mybir
    import concourse.tile as tile

    dt = mybir.dt
    f32 = dt.float32
    bf16 = dt.bfloat16
    fp8 = dt.float8e4
    Alu = mybir.AluOpType
    Act = mybir.ActivationFunctionType

    imadd = _register_imadd()

    nc = bacc.Bacc("TRN2", target_bir_lowering=False, debug=False,
                   num_devices=N_CORES)

    def tt(out, in0, in1, op):
        return nc.vector.add_instruction(mybir.InstTensorTensor(
            name=nc.get_next_instruction_name(), op=op,
            ins=[nc.vector.lower_ap(in0), nc.vector.lower_ap(in1)],
            outs=[nc.vector.lower_ap(out)]))

    # ---- DRAM I/O ----
    dram_in = {}

    def din(name, shape, dtp):
        dram_in[name] = nc.dram_tensor(name, list(shape), dtp,
                                       kind="ExternalInput").ap()

    for m in ("h", "w"):
        din(f"tokT_{m}", (CIO, TOK), bf16)
        din(f"winT_{m}", (CIO, 4 * D_IN), bf16)   # fused (in_w@down).T
        din(f"convw_{m}", (D_INT, KCV), f32)
        din(f"convb_{m}", (D_INT, 1), f32)
        din(f"wdT_{m}", (D_INT, D_INT), bf16)     # (dproj@xproj[:6]).T
        din(f"dpb_{m}", (D_INT, 1), f32)
        din(f"xbcT_{m}", (D_INT, 2 * NST), bf16)  # xproj[6:].T  [B|C]
        din(f"A_{m}", (D_INT, NST), f32)          # -exp(clip(A_log))
        din(f"Dp_{m}", (D_INT, 1), f32)
        din(f"woutT_{m}", (D_INT, CIO), bf16)     # (bn_inv*(up@out_w)).T
        din(f"bout_{m}", (CIO, 1), f32)

    dram_out = {
        "h": nc.dram_tensor("out_h", [CIO, TOK], f32,
                            kind="ExternalOutput").ap(),
        "w": nc.dram_tensor("out_w", [CIO, TOK], f32,
                            kind="ExternalOutput").ap(),
    }

    HSP, TSP = 128, 64          # d split: head rows / tail rows
    with tile.TileContext(nc) as tc:
        with (
            tc.tile_pool(name="wts", bufs=1) as wts,
            tc.tile_pool(name="sm", bufs=1) as sm,
            tc.tile_pool(name="big", bufs=1) as big,
            tc.tile_pool(name="big2", bufs=1) as big2,
            tc.tile_pool(name="psA", bufs=1, space="PSUM") as psA,
            tc.tile_pool(name="psB", bufs=2, space="PSUM") as psB,
        ):
            ones8 = wts.tile([HSP, HSP], fp8, name="ones8")
            nc.gpsimd.memset(ones8[:], 1.0)

            ROWS = (HSP, TSP)

            def halved(name_base, m, cols, dtp):
                out = []
                for hf in range(2):
                    r0 = hf * HSP
                    t = wts.tile([ROWS[hf], cols], dtp,
                                 name=f"{name_base}{hf}_{m}",
                                 tag=f"{name_base}{hf}")
                    nc.sync.dma_start(
                        t[:], dram_in[f"{name_base}_{m}"][r0:r0 + ROWS[hf], :])
                    out.append(t)
                return out

            # normal-layout view helpers: [rows, 784] -> [rows, 7, 56]
            def norm_q(t784, q):
                return t784.rearrange("p (g q t) -> p g q t",
                                      q=2, t=L)[:, :, q]

            def int_p(t784, p):
                return t784.rearrange("p (g t q) -> p g t q",
                                      t=L, q=2)[:, :, :, p]

            for m in ("h", "w"):
                # ---------- load weights ----------
                tokT = wts.tile([CIO, TOK], bf16, name=f"tokT_{m}",
                                tag="tokT")
                nc.sync.dma_start(tokT[:], dram_in[f"tokT_{m}"][:])
                winT = wts.tile([CIO, 4 * D_IN], bf16, name=f"winT_{m}",
                                tag="winT")
                nc.sync.dma_start(winT[:], dram_in[f"winT_{m}"][:])
                convw = halved("convw", m, KCV, f32)
                convb = halved("convb", m, 1, f32)
                wdT = halved("wdT", m, D_INT, bf16)
                dpb = halved("dpb", m, 1, f32)
                xbcT = halved("xbcT", m, 2 * NST, bf16)
                Amat = halved("A", m, NST, f32)
                Dp = halved("Dp", m, 1, f32)
                woutT = halved("woutT", m, CIO, bf16)
                bout = wts.tile([CIO, 1], f32, name=f"bout_{m}", tag="bout")
                nc.sync.dma_start(bout[:], dram_in[f"bout_{m}"][:])
                # A rows for tail blocks: tail half duplicated on both halves
                Adup = wts.tile([HSP, NST], f32, name=f"Adup_{m}", tag="Adup")
                nc.sync.dma_start(Adup[0:TSP, :], dram_in[f"A_{m}"][HSP:, :])
                nc.sync.dma_start(Adup[TSP:, :], dram_in[f"A_{m}"][HSP:, :])

                # ---------- in-projection (fused down-proj) ----------
                FCH = ((0, 0), (1, HSP), (2, D_INT), (3, D_INT + HSP))
                x1pad, x1s, res_s = [None, None], [None, None], [None, None]
                for hf in range(2):
                    xp = sm.tile([ROWS[hf], PADC], bf16,
                                 name=f"x1pad{hf}_{m}", tag=f"x1pad{hf}")
                    nc.gpsimd.memset(xp[:], 0.0)
                    x1pad[hf] = xp
                for fc in range(4):
                    hf = fc % 2
                    col0 = FCH[fc][1]
                    rows = ROWS[hf]
                    ps = psA.tile([rows, TOK], f32, name=f"psin{fc}_{m}",
                                  tag=f"psA{hf}")
                    for c0 in range(0, TOK, 512):
                        c1 = min(c0 + 512, TOK)
                        nc.tensor.matmul(ps[:, c0:c1],
                                         winT[:, col0:col0 + rows],
                                         tokT[:, c0:c1],
                                         start=True, stop=True)
                    if fc < 2:
                        dst = x1pad[hf][:, 4:4 + SPC * PITCH].rearrange(
                            "p (s t) -> p s t", t=PITCH)[:, :, 0:L]
                        nc.scalar.copy(dst,
                                       ps.rearrange("p (s t) -> p s t", t=L))
                    else:
                        rs = sm.tile([rows, TOK], bf16, name=f"res{hf}_{m}",
                                     tag=f"res{hf}")
                        nc.scalar.activation(rs[:], ps[:], Act.Silu)
                        res_s[hf] = rs

                # ---------- depthwise causal conv + SiLU ----------
                for hf in range(2):
                    rows = ROWS[hf]
                    ca = sm.tile([rows, TOK], bf16, name=f"ca{hf}_{m}",
                                 tag=f"ca{hf}")
                    cb = sm.tile([rows, TOK], bf16, name=f"cb{hf}_{m}",
                                 tag=f"cb{hf}")

                    def tap(k, _hf=hf):
                        return x1pad[_hf][:, 1 + k:1 + k +
                                          SPC * PITCH].rearrange(
                            "p (s t) -> p s t", t=PITCH)[:, :, 0:L]

                    ca3 = ca.rearrange("p (s t) -> p s t", t=L)
                    cb3 = cb.rearrange("p (s t) -> p s t", t=L)
                    nc.vector.tensor_scalar_mul(ca3, tap(0), convw[hf][:, 0:1])
                    nc.vector.scalar_tensor_tensor(cb3, tap(1),
                                                   convw[hf][:, 1:2], ca3,
                                                   op0=Alu.mult, op1=Alu.add)
                    nc.vector.scalar_tensor_tensor(ca3, tap(2),
                                                   convw[hf][:, 2:3], cb3,
                                                   op0=Alu.mult, op1=Alu.add)
                    nc.vector.scalar_tensor_tensor(cb3, tap(3),
                                                   convw[hf][:, 3:4], ca3,
                                                   op0=Alu.mult, op1=Alu.add)
                    xs = sm.tile([rows, TOK], bf16, name=f"x1s{hf}_{m}",
                                 tag=f"x1s{hf}")
                    nc.scalar.activation(xs[:], cb[:], Act.Silu,
                                         bias=convb[hf][:])
                    x1s[hf] = xs

                # ---------- x_dbl: delta (pair-interleaved) / B / C -------
                dlI, ppI = [None, None], [None, None]
                for hf in range(2):
                    rows = ROWS[hf]
                    ps = psA.tile([rows, TOK], f32, name=f"psd{hf}_{m}",
                                  tag=f"psA{hf}")
                    col0 = hf * HSP
                    for c0 in range(0, TOK, 512):
                        c1 = min(c0 + 512, TOK)
                        nc.tensor.matmul(ps[:, c0:c1],
                                         wdT[0][:, col0:col0 + rows],
                                         x1s[0][:, c0:c1],
                                         start=True, stop=False)
                        nc.tensor.matmul(ps[:, c0:c1],
                                         wdT[1][:, col0:col0 + rows],
                                         x1s[1][:, c0:c1],
                                         start=False, stop=True)
                    dl = sm.tile([rows, TOK], bf16, name=f"dlI{hf}_{m}",
                                 tag=f"delta{hf}")
                    dtmp = sm.tile([rows, TOK], bf16, name=f"dtmp{hf}_{m}",
                                   tag=f"P{hf}")
                    nc.vector.tensor_scalar_min(dtmp[:], ps[:], 30.0)
                    nc.scalar.activation(dl[:], dtmp[:], Act.Exp,
                                         bias=dpb[hf][:])
                    nc.vector.tensor_scalar_add(dtmp[:], dl[:], 1.0)
                    # final softplus Ln, written PAIR-INTERLEAVED
                    for p in range(2):
                        nc.scalar.activation(int_p(dl, p), norm_q(dtmp, p),
                                             Act.Ln)
                    dlI[hf] = dl

                Bsb = sm.tile([NST, TOK], fp8, name=f"Bsb_{m}", tag="Bsb")
                Csb = sm.tile([NST, TOK], fp8, name=f"Csb_{m}", tag="Csb")
                for bc in range(2):
                    ps = psA.tile([NST, TOK], f32, name=f"psbc{bc}_{m}",
                                  tag=f"psA{bc}")
                    for c0 in range(0, TOK, 512):
                        c1 = min(c0 + 512, TOK)
                        nc.tensor.matmul(ps[:, c0:c1],
                                         xbcT[0][:, bc * NST:(bc + 1) * NST],
                                         x1s[0][:, c0:c1],
                                         start=True, stop=False)
                        nc.tensor.matmul(ps[:, c0:c1],
                                         xbcT[1][:, bc * NST:(bc + 1) * NST],
                                         x1s[1][:, c0:c1],
                                         start=False, stop=True)
                    nc.scalar.copy((Bsb if bc == 0 else Csb)[:], ps[:])

                # ---------- P = delta*u, pair-interleaved ----------
                for hf in range(2):
                    rows = ROWS[hf]
                    pp = sm.tile([rows, TOK], bf16, name=f"ppI{hf}_{m}",
                                 tag=f"P{hf}")
                    for p in range(2):
                        nc.vector.scalar_tensor_tensor(
                            int_p(pp, p), int_p(dlI[hf], p), 1.0,
                            norm_q(x1s[hf], p), op0=Alu.mult, op1=Alu.mult)
                    ppI[hf] = pp

                # ---------- per-seq B|C flats (fp8), gathered lazily ------
                # two tiles, rows at partitions 0/64 (matmul base-partition
                # rule): 4 rotation slots for the 4 in-flight sequences.
                flA = sm.tile([HSP, 2 * BIG], fp8, name=f"flA_{m}",
                              tag="flA")
                flB = sm.tile([HSP, 2 * BIG], fp8, name=f"flB_{m}",
                              tag="flB")
                flats_done = set()

                def get_flat(s):
                    tl = flA if s % 4 < 2 else flB
                    r = TSP * (s % 2)
                    if s not in flats_done:
                        nc.sync.dma_start(
                            tl[r:r + 1, 0:BIG].rearrange(
                                "p (n t) -> p n t", t=L),
                            Bsb.rearrange("n (s t) -> n s t", t=L)[:, s])
                        nc.sync.dma_start(
                            tl[r:r + 1, BIG:].rearrange(
                                "p (n t) -> p n t", t=L),
                            Csb.rearrange("n (s t) -> n s t", t=L)[:, s])
                        flats_done.add(s)
                    return tl[r:r + 1]

                # ---------- A tiles broadcast over tp, with reset poison --
                def mk_aexp(src, name):
                    t = wts.tile([HSP, CPB], bf16, name=name, tag=name[:5])
                    t3 = t.rearrange("p (n c) -> p n c", c=TP)
                    nc.scalar.copy(t3, src.unsqueeze(2).to_broadcast(
                        [HSP, NST, TP]))
                    nc.gpsimd.memset(t3[:, :, 0:2], -300.0)
                    return t

                aexpIh = mk_aexp(Amat[0], f"aexph_{m}")
                aexpId = mk_aexp(Adup, f"aexpd_{m}")

                # y accumulators (interleaved layout)
                yIh = sm.tile([HSP, TOK], bf16, name=f"yIh_{m}", tag="cb0")
                yIt = sm.tile([TSP, TOK], bf16, name=f"yIt_{m}", tag="cb1")

                # ---------- scan blocks ----------
                def make_av(aexp, dv, rows):
                    av = big.tile([HSP, CPB], bf16, name="av", tag="s2",
                                  bufs=2)
                    avs = av[0:rows]
                    tt(avs.rearrange("p (n c) -> p n c", c=TP),
                       aexp[0:rows].rearrange("p (n c) -> p n c", c=TP),
                       dv.unsqueeze(1).to_broadcast([rows, NST, TP]),
                       Alu.mult)
                    nc.scalar.activation(avs[:], avs[:], Act.Exp)
                    return av

                def main_block(av, pv, reps, rows, ydst, av_hook=None):
                    NCH = 16
                    CW = NCH * TP     # 1792 cols per replication chunk

                    def replicate(kind, n0, tag, bufs):
                        rep_t = big2.tile([HSP, CW], bf16,
                                          name=tag, tag=tag, bufs=bufs)
                        off = kind * BIG
                        for p in range(2):
                            psb = psB.tile([HSP, NCH * L], f32,
                                           name=f"ps{tag}", tag="psb")
                            for pairidx, r0, r1 in reps:
                                s = 2 * pairidx + p
                                fl = get_flat(s)
                                fb = TSP * (s % 2)
                                base = off + n0 * L
                                for q0 in range(0, NCH * L, 512):
                                    q1 = min(q0 + 512, NCH * L)
                                    nc.tensor.matmul(
                                        psb[r0:r1, q0:q1],
                                        ones8[fb:fb + 1, 0:r1 - r0],
                                        fl[0:1, base + q0:base + q1],
                                        start=True, stop=True)
                            dst = rep_t.rearrange(
                                "p (n t q) -> p n t q",
                                t=L, q=2)[0:rows, :, :, p]
                            nc.scalar.copy(
                                dst,
                                psb[0:rows].rearrange("p (n t) -> p n t",
                                                      t=L))
                        return rep_t

                    bv = big.tile([HSP, CPB], bf16, name="bv", tag="s4")
                    bvs = bv[0:rows]
                    bv3 = bvs.rearrange("p (n c) -> p n c", c=TP)
                    for n0 in range(0, NST, NCH):
                        brep = replicate(0, n0, "brep", 3)
                        tt(bv3[:, n0:n0 + NCH],
                           pv.unsqueeze(1).to_broadcast([rows, NCH, TP]),
                           brep[0:rows].rearrange("p (n c) -> p n c", c=TP),
                           Alu.mult)

                    hook_av = av_hook() if av_hook else None

                    creps = []
                    for n0 in range(0, NST, NCH):
                        creps.append(replicate(1, n0, "crep", 6))

                    # in-place interleaved madd scan: bv <- scan(av, bv)
                    nc.vector._custom_dve(imadd, out=bvs[:], in0=av[0:rows],
                                          in1=bvs[:])
                    # zc in place: bv <- bv * crep
                    for ci, n0 in enumerate(range(0, NST, NCH)):
                        tt(bvs[:, n0 * TP:(n0 + NCH) * TP],
                           bvs[:, n0 * TP:(n0 + NCH) * TP],
                           creps[ci][0:rows], Alu.mult)
                    nh = NST
                    while nh > 3:
                        nh //= 2
                        tt(bvs[:, 0:nh * TP], bvs[:, 0:nh * TP],
                           bvs[:, nh * TP:2 * nh * TP], Alu.add)
                    nc.vector.scalar_tensor_tensor(
                        ydst, bvs[:, 0:TP], 1.0, bvs[:, TP:2 * TP],
                        op0=Alu.mult, op1=Alu.add)
                    nc.vector.scalar_tensor_tensor(
                        ydst, ydst, 1.0, bvs[:, 2 * TP:3 * TP],
                        op0=Alu.mult, op1=Alu.add)
                    return hook_av

                # descriptors: g0 head, g1 head, quad(g0,g1), ...
                # each entry is a prep closure run one block ahead of use,
                # emitting its DMAs and returning the block's operands.
                def head_prep(g):
                    def _p():
                        gcols = slice(g * TP, (g + 1) * TP)
                        return dict(
                            aexp=aexpIh, dv=dlI[0][:, gcols],
                            pv=ppI[0][:, gcols], reps=[(g, 0, HSP)],
                            rows=HSP, ydst=yIh[:, gcols], post=None)
                    return _p

                def quad_prep(q):
                    def _p():
                        dq = sm.tile([HSP, TP], bf16, name=f"dq{q}_{m}",
                                     tag="dq", bufs=3)
                        pq = sm.tile([HSP, TP], bf16, name=f"pq{q}_{m}",
                                     tag="pq", bufs=3)
                        c0 = (2 * q) * TP
                        c1 = (2 * q + 1) * TP
                        nc.sync.dma_start(dq[0:TSP], dlI[1][:, c0:c0 + TP])
                        nc.sync.dma_start(dq[TSP:], dlI[1][:, c1:c1 + TP])
                        nc.sync.dma_start(pq[0:TSP], ppI[1][:, c0:c0 + TP])
                        nc.sync.dma_start(pq[TSP:], ppI[1][:, c1:c1 + TP])
                        ytq = sm.tile([HSP, TP], bf16, name=f"ytq{q}_{m}",
                                      tag="ytq", bufs=2)
                        return dict(
                            aexp=aexpId, dv=dq[:], pv=pq[:],
                            reps=[(2 * q, 0, TSP), (2 * q + 1, TSP, HSP)],
                            rows=HSP, ydst=ytq[:], post=(ytq, c0, c1))
                    return _p

                def tail6_prep():
                    g6 = slice(6 * TP, 7 * TP)
                    return dict(
                        aexp=aexpId, dv=dlI[1][:, g6], pv=ppI[1][:, g6],
                        reps=[(6, 0, TSP)], rows=TSP,
                        ydst=yIt[:, g6], post=None)

                preps = []
                for g in range(NPAIR):
                    preps.append(head_prep(g))
                    if g % 2 == 1:
                        preps.append(quad_prep(g // 2))
                preps.append(tail6_prep)

                descs = [preps[0]()]
                av_next = make_av(descs[0]["aexp"], descs[0]["dv"],
                                  descs[0]["rows"])
                for i in range(len(preps)):
                    dsc = descs[i]
                    av_cur = av_next
                    if i + 1 < len(preps):
                        descs.append(preps[i + 1]())
                        nd = descs[i + 1]
                        hook = (lambda nd=nd: make_av(nd["aexp"], nd["dv"],
                                                      nd["rows"]))
                    else:
                        hook = None
                    av_next = main_block(av_cur, dsc["pv"], dsc["reps"],
                                         dsc["rows"], dsc["ydst"],
                                         av_hook=hook)
                    if dsc["post"] is not None:
                        ytq, c0, c1 = dsc["post"]
                        nc.sync.dma_start(yIt[:, c0:c0 + TP], ytq[0:TSP])
                        nc.sync.dma_start(yIt[:, c1:c1 + TP], ytq[TSP:])

                # ---------- de-interleave y ----------
                yt = [None, None]
                for hf, srcI in ((0, yIh), (1, yIt)):
                    y = sm.tile([ROWS[hf], TOK], bf16, name=f"y{hf}_{m}",
                                tag=f"y{hf}")
                    for p in range(2):
                        nc.scalar.copy(norm_q(y, p), int_p(srcI, p))
                    yt[hf] = y

                # ---------- epilogue ----------
                rr = [None, None]
                for hf in range(2):
                    rows = ROWS[hf]
                    y2 = sm.tile([rows, TOK], bf16, name=f"y2{hf}_{m}",
                                 tag=f"P{hf}")
                    nc.vector.scalar_tensor_tensor(y2[:], x1s[hf][:],
                                                   Dp[hf][:], yt[hf][:],
                                                   op0=Alu.mult, op1=Alu.add)
                    r = sm.tile([rows, TOK], bf16, name=f"rr{hf}_{m}",
                                tag=f"delta{hf}")
                    nc.vector.scalar_tensor_tensor(r[:], y2[:], 1.0,
                                                   res_s[hf][:],
                                                   op0=Alu.mult, op1=Alu.mult)
                    rr[hf] = r
                pso = psA.tile([CIO, TOK], f32, name=f"pso_{m}", tag="psA0")
                for c0 in range(0, TOK, 512):
                    c1 = min(c0 + 512, TOK)
                    nc.tensor.matmul(pso[:, c0:c1], woutT[0][:],
                                     rr[0][:, c0:c1], start=True, stop=False)
                    nc.tensor.matmul(pso[:, c0:c1], woutT[1][:],
                                     rr[1][:, c0:c1], start=False, stop=True)
                ot = sm.tile([CIO, TOK], f32, name=f"ot_{m}", tag="x1pad0")
                nc.scalar.activation(ot[:], pso[:], Act.Identity,
                                     bias=bout[:])
                nc.sync.dma_start(dram_out[m][:], ot[:])

    nc.compile()
    return nc


def _host_prep(inputs):
    """Fuse weights on host (tiny), build per-core input maps."""
    import ml_dtypes
    bf16 = ml_dtypes.bfloat16

    def f(k):
        return np.asarray(inputs[k], np.float32)

    x = f("x")
    maps_common = {}
    for m, dn, up, gm, bt, mn, vr in (
        ("h", "hd_w", "hu_w", "hn_gamma", "hn_beta", "hn_mean", "hn_var"),
        ("w", "wd_w", "wu_w", "wn_gamma", "wn_beta", "wn_mean", "wn_var"),
    ):
        p = "hm_" if m == "h" else "wm_"
        in_w = f(p + "in_w")
        conv_w = f(p + "conv_w")
        conv_b = f(p + "conv_b")
        xproj = f(p + "xproj_w")
        dpw = f(p + "dproj_w")
        dpbv = f(p + "dproj_b")
        A_log = f(p + "A_log")
        Dv = f(p + "D")
        out_w = f(p + "out_w")
        dnw = f(dn)
        upw = f(up)
        inv = f(gm) / np.sqrt(f(vr) + np.float32(BN_EPS))
        maps_common[f"winT_{m}"] = np.ascontiguousarray(
            (in_w @ dnw).T).astype(bf16)
        maps_common[f"convw_{m}"] = np.ascontiguousarray(conv_w[:, 0, :])
        maps_common[f"convb_{m}"] = np.ascontiguousarray(conv_b[:, None])
        maps_common[f"wdT_{m}"] = np.ascontiguousarray(
            (dpw @ xproj[:DTR]).T).astype(bf16)
        maps_common[f"dpb_{m}"] = np.ascontiguousarray(dpbv[:, None])
        maps_common[f"xbcT_{m}"] = np.ascontiguousarray(
            xproj[DTR:].T).astype(bf16)
        maps_common[f"A_{m}"] = np.ascontiguousarray(
            -np.exp(np.clip(A_log, -5.0, 5.0)))
        maps_common[f"Dp_{m}"] = np.ascontiguousarray(Dv[:, None])
        wo = inv[:, None] * (upw @ out_w)
        maps_common[f"woutT_{m}"] = np.ascontiguousarray(wo.T).astype(bf16)
        maps_common[f"bout_{m}"] = np.ascontiguousarray(
            (f(bt) - f(mn) * inv)[:, None])

    # token matrices, channel-major:  h: (c, b, w, h)   w: (c, b, h, w)
    seq_h = np.ascontiguousarray(
        x.transpose(1, 0, 3, 2).reshape(CIO, B * WW * HH))
    seq_w = np.ascontiguousarray(
        x.transpose(1, 0, 2, 3).reshape(CIO, B * HH * WW))
    in_maps = []
    for c in range(N_CORES):
        mp = dict(maps_common)
        mp["tokT_h"] = np.ascontiguousarray(
            seq_h[:, c * TOK:(c + 1) * TOK]).astype(bf16)
        mp["tokT_w"] = np.ascontiguousarray(
            seq_w[:, c * TOK:(c + 1) * TOK]).astype(bf16)
        in_maps.append(mp)
    return in_maps


_NP_BIN = ("/nix/store/9glay7jc4kbsam83g8wdzrwcmfcygwx5-neuron-env/bin/"
           "neuron-profile")


def _profile_exec_ns(nc, in_maps):
    """Capture an NTFF profile of one SPMD execute via the axon sidechannel
    and return the kernel's on-device total execution time in ns."""
    import ctypes
    import glob
    import json
    import shutil
    import subprocess
    import tempfile

    from concourse import bass2jax

    try:
        lib = ctypes.CDLL("/opt/axon/libaxon_pjrt.so")
        if not hasattr(lib, "axon_start_nrt_profile"):
            return None
        lib.axon_start_nrt_profile.argtypes = [
            ctypes.POINTER(ctypes.c_int64), ctypes.c_size_t]
        lib.axon_start_nrt_profile.restype = ctypes.c_int64
        lib.axon_stop_nrt_profile.argtypes = [ctypes.c_char_p]
        lib.axon_stop_nrt_profile.restype = ctypes.c_int64

        best = None
        for _ in range(int(os.environ.get("KPROF_N", "3"))):
            prof_dir = tempfile.mkdtemp(prefix="ntff_")
            ids = (ctypes.c_int64 * 1)(0)
            if lib.axon_start_nrt_profile(ids, 1) != 0:
                return best
            try:
                bass2jax.run_bass_via_pjrt(nc, in_maps, n_cores=N_CORES)
            finally:
                nfiles = lib.axon_stop_nrt_profile(prof_dir.encode())
            if nfiles <= 0:
                continue
            ntffs = sorted(glob.glob(os.path.join(prof_dir, "*.ntff")))
            neffs = sorted(glob.glob(os.path.join(prof_dir, "*.neff")))
            if not ntffs or not neffs:
                continue
            out = subprocess.run(
                [_NP_BIN, "view", "-n", neffs[-1], "-s", ntffs[-1],
                 "--output-format", "summary-json"],
                capture_output=True, text=True, timeout=300)
            data = json.loads(out.stdout)
            for v in data.values():
                if isinstance(v, dict) and "total_time" in v:
                    t = int(float(v["total_time"]) * 1e9)
                    best = t if best is None else min(best, t)
            shutil.rmtree(prof_dir, ignore_errors=True)
        return best
    except Exception:
        return None
    return None


def kernel(x, **kw):
    global LAST_HW_EXEC_NS
    inputs = dict(kw)
    inputs["x"] = x
    if "nc" not in _CACHE:
        _CACHE["nc"] = _build_bass()
    nc = _CACHE["nc"]

    from concourse import bass2jax

    in_maps = _host_prep(inputs)
    results = bass2jax.run_bass_via_pjrt(nc, in_maps, n_cores=N_CORES)

    if os.environ.get("KPROF", "1") == "1" and _CACHE.get("prof_ns") is None:
        _CACHE["prof_ns"] = _profile_exec_ns(nc, in_maps)
    if _CACHE.get("prof_ns"):
        LAST_HW_EXEC_NS = int(_CACHE["prof_ns"])

    xf = np.asarray(x, np.float32)
    h_cols = np.concatenate([results[c]["out_h"] for c in range(N_CORES)],
                            axis=1)
    w_cols = np.concatenate([results[c]["out_w"] for c in range(N_CORES)],
                            axis=1)
    h_full = h_cols.reshape(CIO, B, WW, HH).transpose(1, 0, 3, 2)
    w_full = w_cols.reshape(CIO, B, HH, WW).transpose(1, 0, 2, 3)
    return (h_full + w_full + xf).astype(np.float32)
